# revision 7
# baseline (speedup 1.0000x reference)
"""Trainium2 Bass kernel for nn_HardMemory (retrieval_knn).

For each spatial token (B*H*W tokens, C=128 channels), find the memory row
(of M=512) with max cosine similarity and replace the token's channel vector
with that raw memory row.

Algebraic simplification: argmax_m cos(x, mem_m) = argmax_m (x . mem_n_m)
where mem_n is the l2-normalized memory -- normalizing x is a positive
per-token scale and cannot change the argmax, so it is skipped.

Scores (PSUM fp32, beta-scaled): s = xh.mh16 (one fp16 matmul) plus two fp8
DoubleRow matmuls carrying the precision cross-terms at half cost:
  DR-A: (xl1, xl2).(mhA, mhB)   ~= xl.mh      (xl = fp16 residual of x,
        split in two e4m3 terms pre-scaled by 2^16; mh slots down-scaled)
  DR-B: (xh8a, xh8b).(mlA, mlB) ~= xh.ml      (ml = fp16 residual of the
        beta-scaled memory; xh split in two e4m3 terms)
Each DoubleRow sums TWO independent 128-deep products at 0.5 cycles/row, so
per-tile score cost is 512+256+256 = 1024 PE cycles vs 1536 for the 3-term
fp16 scheme. Dropped terms are O(2^-15) of score scale; measured ~10 argmax
flips over 131072 tokens, well under the 2e-2 rel-err gate.

One-hot: exp(s - max) on ACT (bias = negated reduce_max, scale=1), written
as e4m3 bytes at stride 2 into a SHARED fp16 tile: tile j of the group owns
byte plane j. One fp16 128x128 transpose then moves BOTH tiles' one-hot
bytes at once (one-hot bytes are <= 0x38 so the fp16 lanes are always
finite positive and the identity-matmul transpose is exact). This halves
both the PE transpose cost and the PSUM->SBUF copy volume vs per-tile fp8
transposes.

The gather reconstructs memory rows from a 2-term fp8 (e4m3 hi+lo) split
via fp8 DoubleRow one-hot matmuls reading stride-2 byte views of the
transposed pack: per tile 4 DR matmuls (2 chunk-pairs x hi/lo) at 0.5
cycles/row. Output fp32 PSUM -> fp16 SBUF copy -> DMA, stored fp16 and
upcast on the host (fp8 recon err ~1e-3 rel, far under the gate).

Engine balance per group (2 tiles, 256 tokens), ~64 groups per core:
  PE  : scores 853ns + transposes 213ns + gather 213ns  = 1280ns
  DVE : batched reduce_max (negate)                     = 1192ns (+ a few
        out-copies)
  ACT : 2 exps                                          = 1224ns
  Pool: packed ohT copy 806ns + most out-copies ~450ns  = ~1210ns
All four engines sit at 76-82us of ~87us span.

Sharding: data-parallel over batch, 4 batches per core, memory replicated.
Input DMAs are sliced and spread across groups so the 625ns HWDGE
descriptor setups never serialize against compute.
"""

import numpy as np

import concourse.bass as bass
import concourse.mybir as mybir
from concourse.tile import TileContext
from concourse.bass_utils import run_bass_kernel_spmd

F32 = mybir.dt.float32
F16 = mybir.dt.float16
F8 = mybir.dt.float8e4
AF = mybir.ActivationFunctionType

B, C, H, W = 32, 128, 64, 64
N = H * W              # 4096 tokens per batch
M = 512                # memory rows
NCORES = 8
BPC = B // NCORES      # batches per core
TOK = BPC * N          # tokens per core
TILE = 128             # tokens per tile
GRP = 2                # tiles per PSUM score group
LOAD = 4096            # tokens per input DMA region (one full batch image)
STORE = 512            # tokens per output DMA chunk (2 groups)
KCH = M // TILE        # 4 gather chunks
SIG = 65536.0          # xl pre-scale for the fp8 split (exact power of 2)


def _build():
    nc = bass.Bass(trn_type="TRN2")

    xh_in = nc.dram_tensor("xh", [BPC, C, N], F16, kind="ExternalInput")
    # fp8 slots: 0=xl1 1=xl2 (DR-A lhsT pair), 2=xh8a 3=xh8b (DR-B pair)
    x8_in = nc.dram_tensor("x8", [BPC, C, 4, N], F8, kind="ExternalInput")
    mh_in = nc.dram_tensor("mh16", [C, M], F16, kind="ExternalInput")
    # fp8 slots: 0=mhA 1=mhB (DR-A rhs pair), 2=mlA 3=mlB (DR-B rhs pair)
    m8_in = nc.dram_tensor("m8", [C, 4, M], F8, kind="ExternalInput")
    # raw memory rows fp8 hi/lo [TILE, KCH, 2, C] packed
    g8_in = nc.dram_tensor("g8", [TILE, KCH * 2 * C], F8, kind="ExternalInput")
    idu_in = nc.dram_tensor("idu", [TILE, TILE], F16, kind="ExternalInput")
    out_d = nc.dram_tensor("out", [BPC, C, N], F16, kind="ExternalOutput")

    with TileContext(nc) as tc:
        with (
            tc.tile_pool(name="const", bufs=1) as cpool,
            tc.tile_pool(name="xin", bufs=4) as xpool,
            tc.tile_pool(name="oh", bufs=4) as ohpool,
            tc.tile_pool(name="oht", bufs=4) as ohtpool,
            tc.tile_pool(name="osb", bufs=4) as opool,
            tc.tile_pool(name="small", bufs=6) as spool,
            tc.tile_pool(name="ps_s", bufs=2, space="PSUM") as ps_s,
            tc.tile_pool(name="ps_t", bufs=2, space="PSUM") as ps_t,
            tc.tile_pool(name="ps_o", bufs=2, space="PSUM") as ps_o,
        ):
            n_groups = TOK // (TILE * GRP)
            grp_per_batch = N // (TILE * GRP)
            loaded = {}

            def load_slice(b, s0, s1):
                nb = 8 if s1 - s0 == 1024 else 1
                xh_sb = xpool.tile(
                    [C, s1 - s0], F16, tag=f"xh{s1 - s0}", bufs=nb
                )
                nc.sync.dma_start(out=xh_sb, in_=xh_in[b, :, s0:s1])
                x8_sb = xpool.tile(
                    [C, 4, s1 - s0], F8, tag=f"x8{s1 - s0}", bufs=nb
                )
                nc.sync.dma_start(out=x8_sb, in_=x8_in[b, :, :, s0:s1])
                loaded.setdefault(b, []).append((s0, s1, xh_sb, x8_sb))

            def xslice(b, o, size):
                for s0, s1, xh_sb, x8_sb in loaded[b]:
                    if s0 <= o and o + size <= s1:
                        return (xh_sb[:, o - s0 : o - s0 + size],
                                x8_sb[:, :, o - s0 : o - s0 + size])
                raise AssertionError((b, o, size))

            # Prefetch schedule: batches 1..3 load one 1024-token slice per
            # group, starting 12 groups before the batch is needed.
            load_plan = {}
            for b in range(1, BPC):
                for si in range(4):
                    load_plan.setdefault(
                        b * grp_per_batch - 12 + 2 * si, []
                    ).append((b, si * 1024, (si + 1) * 1024))

            mh16 = cpool.tile([C, M], F16)
            nc.sync.dma_start(out=mh16, in_=mh_in[:])
            m8 = cpool.tile([C, 4, M], F8)
            nc.sync.dma_start(out=m8, in_=m8_in[:])
            # batch 0: small leading slices so PE starts early
            for s0, s1 in ((0, 256), (256, 1024), (1024, 2048), (2048, 3072),
                           (3072, N)):
                load_slice(0, s0, s1)
            g8 = cpool.tile([TILE, KCH * 2 * C], F8)
            nc.sync.dma_start(out=g8, in_=g8_in[:])
            g8v = g8.rearrange("p (k h c) -> p k h c", k=KCH, h=2)
            idu = cpool.tile([TILE, TILE], F16)
            nc.sync.dma_start(out=idu, in_=idu_in[:])

            DR = mybir.MatmulPerfMode.DoubleRow

            def head(g):
                """Score matmuls for group g + batched negated max."""
                gtok0 = g * TILE * GRP
                b = gtok0 // N
                for lb, s0, s1 in load_plan.get(g, ()):
                    load_slice(lb, s0, s1)
                ps4 = ps_s.tile([TILE, GRP, M], F32)
                for j in range(GRP):
                    tok0 = gtok0 + j * TILE
                    o = tok0 % LOAD
                    xht, x8t = xslice(b, o, TILE)
                    ps = ps4[:, j, :]
                    nc.tensor.matmul(out=ps, lhsT=xht, rhs=mh16,
                                     start=True, stop=False)
                    nc.tensor.matmul(out=ps, lhsT=x8t[:, 0:2, :],
                                     rhs=m8[:, 0:2, :],
                                     start=False, stop=False, perf_mode=DR)
                    nc.tensor.matmul(out=ps, lhsT=x8t[:, 2:4, :],
                                     rhs=m8[:, 2:4, :],
                                     start=False, stop=True, perf_mode=DR)
                nbmx = spool.tile([TILE, GRP], F32, tag="nbmx")
                nc.vector.reduce_max(
                    out=nbmx, in_=ps4, axis=mybir.AxisListType.X, negate=True
                )
                return ps4, nbmx

            def exp_stage(g, ps4, nbmx):
                """Exp one-hot for group g (1 group late): both tiles write
                fp8 bytes into one shared fp16-typed pack tile (tile j owns
                byte plane j)."""
                ohp = ohpool.tile([TILE, M], F16)
                oh8 = ohp.bitcast(F8).rearrange("p (m two) -> p m two", two=2)
                for j in range(GRP):
                    nc.scalar.activation(
                        out=oh8[:, :, j], in_=ps4[:, j, :], func=AF.Exp,
                        bias=nbmx[:, j : j + 1], scale=1.0,
                    )
                return ohp

            psT_cur = [None]

            def tr_stage(g, ohp):
                """4 fp16 transposes move both byte planes at once (2 groups
                late). Pairs of groups share one 2-bank PSUM tile; a single
                batched DVE copy (2x fp16 mode) moves both to SBUF.
                (GPSIMD cannot access PSUM, so DVE/ACT carry all copies.)"""
                if g % 2 == 0:
                    psT_tile = ps_t.tile([TILE, 2, KCH, TILE], F16, tag="psT")
                    psT_cur[0] = psT_tile
                psT = psT_cur[0]
                for k in range(KCH):
                    nc.tensor.transpose(
                        out=psT[:, g % 2, k, :],
                        in_=ohp[:, k * TILE : (k + 1) * TILE],
                        identity=idu,
                    )
                if g % 2 == 0:
                    return None
                ohT = ohtpool.tile([TILE, 2, KCH, TILE], F16)
                nc.vector.tensor_copy(ohT, psT)
                return ohT

            po_cur = [None]

            def gather_stage(g, ohT, half):
                """fp8 DoubleRow gather (3 groups late): per tile 4 DR
                matmuls (chunk-pair x hi/lo). Pairs of groups share one
                [C, 512] fp32 PSUM bank."""
                if g % 2 == 0:
                    po_tile = ps_o.tile([C, STORE], F32, tag="po")
                    po_cur[0] = po_tile
                po = po_cur[0]
                ohT8 = ohT.bitcast(F8).rearrange(
                    "p h k (t two) -> p h k t two", two=2
                )
                for j in range(GRP):
                    col0 = (g % 2) * GRP * TILE + j * TILE
                    n_mm = 0
                    for pr in range(KCH // 2):
                        for h in range(2):
                            nc.tensor.matmul(
                                out=po[:, col0 : col0 + TILE],
                                lhsT=g8v[:, 2 * pr : 2 * pr + 2, h, :],
                                rhs=ohT8[:, half, 2 * pr : 2 * pr + 2, :, j],
                                start=(n_mm == 0), stop=(n_mm == 2 * KCH - 1),
                                perf_mode=DR,
                            )
                            n_mm += 1
                return po if g % 2 == 1 else None

            def tail(g, po):
                """Batched out-copy on ACT (5 groups late): [C, 512] fp32
                PSUM -> fp16 SBUF, then one DMA per STORE tokens."""
                gtok0 = g * TILE * GRP
                ob = opool.tile([C, STORE], F16, tag="ob")
                nc.scalar.activation(out=ob, in_=po, func=AF.Copy)
                b, n0 = divmod(gtok0 + GRP * TILE - STORE, N)
                nc.sync.dma_start(out=out_d[b, :, n0 : n0 + STORE], in_=ob)

            # 6-stage software pipeline: head(g) | exp(g-1) | tr(g-2) |
            # gather(g-4) | tail(g-5). Every cross-engine dependency is >=1
            # full group-iteration old when its consumer reaches it (the
            # 2-group-batched ohT copy lands at pair-iteration 2p+3; the
            # first gather of that pair runs at 2p+4).
            p_head = p_exp = None
            ohts = {}
            pos = {}
            for g in range(n_groups + 6):
                nxt_head = head(g) if g < n_groups else None
                nxt_exp = exp_stage(g - 1, *p_head) if p_head is not None else None
                if p_exp is not None:
                    ohT = tr_stage(g - 2, p_exp)
                    if ohT is not None:
                        ohts[(g - 2) // 2] = ohT
                g4 = g - 4
                if 0 <= g4 < n_groups and (g4 // 2) in ohts:
                    po = gather_stage(g4, ohts[g4 // 2], half=g4 % 2)
                    if po is not None:
                        pos[g4 // 2] = po
                        del ohts[g4 // 2]
                g5 = g - 5
                if g5 >= 0 and g5 % 2 == 1 and (g5 // 2) in pos:
                    tail(g5, pos.pop(g5 // 2))
                p_head, p_exp = nxt_head, nxt_exp

    _legalize_waits(nc)
    nc.finalize()
    return nc


def _legalize_waits(nc):
    """This container's walrus accepts only ONE sync wait per engine
    instruction (setupSyncWait: 'Too many sync wait commands'). Tile emits
    multi-wait instructions (and an 11-wait tail drain). Split: keep one
    wait on the instruction, hoist the rest onto single-wait Drain ops
    inserted just before it on the same engine (engine order preserved =>
    semantics preserved). DMA copies are left alone (ring descriptors
    accept multiple waits)."""
    n_split = 0
    for f in nc.m.functions:
        for b in f.blocks:
            out = []
            for inst in b.instructions:
                si = inst.sync_info
                if si is not None and len(si.on_wait) > 1:
                    waits = list(si.on_wait)
                    for j, w in enumerate(waits[:-1]):
                        out.append(
                            mybir.InstDrain(
                                name=f"{inst.name}-w{j}",
                                engine=inst.engine,
                                ins=[],
                                outs=[],
                                sync_info=mybir.SyncInfo(
                                    on_wait=[w], on_update=[]
                                ),
                            )
                        )
                    inst.sync_info = mybir.SyncInfo(
                        on_wait=[waits[-1]], on_update=list(si.on_update)
                    )
                    n_split += 1
                out.append(inst)
            b.instructions = out
    return n_split


_NC = None


def _get_nc():
    global _NC
    if _NC is None:
        _NC = _build()
    return _NC


def _host_prep(x, memory):
    import ml_dtypes
    f8 = ml_dtypes.float8_e4m3

    memn = memory / np.maximum(
        np.sqrt((memory * memory).sum(axis=1, keepdims=True)), 1e-12
    )
    # Scale the normalized memory by BETA so PSUM holds beta*s directly
    # (exp sharpness K_eff = beta ~ 1e5). Cap keeps fp16 mh finite.
    beta = min(1e5, 55000.0 / max(float(np.abs(memn).max()), 1e-6))
    mnt = np.ascontiguousarray(memn.T).astype(np.float32) * beta   # [C, M]
    mh = mnt.astype(np.float16)
    ml = (mnt - mh.astype(np.float32)).astype(np.float32)

    # DR rhs slot tables (all exact power-of-2 shifts of fp8 encodings)
    mhf = mh.astype(np.float32)
    m8 = np.zeros((C, 4, M), dtype=f8)
    m8[:, 0, :] = (mhf / SIG).astype(f8)             # mhA
    m8[:, 1, :] = (mhf / (SIG * 16.0)).astype(f8)    # mhB
    m8[:, 2, :] = ml.astype(f8)                      # mlA
    m8[:, 3, :] = (ml / 16.0).astype(f8)             # mlB

    xh = x.astype(np.float16)
    xl = (x - xh.astype(np.float32)).astype(np.float32)
    xl1 = (xl * SIG).astype(f8)
    xl2 = ((xl * SIG - xl1.astype(np.float32)) * 16.0).astype(f8)
    xh8a = xh.astype(np.float32).astype(f8)
    xh8b = ((xh.astype(np.float32) - xh8a.astype(np.float32)) * 16.0).astype(f8)
    x8 = np.stack([xl1, xl2, xh8a, xh8b], axis=2)    # [B, C, 4, HW...]

    gh8 = memory.astype(f8)
    gl8 = (memory - gh8.astype(np.float32)).astype(f8)
    g8 = np.zeros((TILE, KCH * 2 * C), dtype=f8)
    for k in range(KCH):
        base = k * 2 * C
        g8[:, base : base + C] = gh8[k * TILE : (k + 1) * TILE, :]
        g8[:, base + C : base + 2 * C] = gl8[k * TILE : (k + 1) * TILE, :]

    return xh, x8, mh, m8, g8


def kernel(x, memory):
    x = np.asarray(x, dtype=np.float32)
    memory = np.asarray(memory, dtype=np.float32)
    nc = _get_nc()
    xf = x.reshape(B, C, N)
    xh, x8, mh16, m8, g8 = _host_prep(xf, memory)
    idu = np.eye(TILE, dtype=np.float16)

    in_maps = []
    for c in range(NCORES):
        in_maps.append({
            "xh": np.ascontiguousarray(xh[c * BPC : (c + 1) * BPC]),
            "x8": np.ascontiguousarray(x8[c * BPC : (c + 1) * BPC]),
            "mh16": mh16, "m8": m8, "g8": g8, "idu": idu,
        })

    res = run_bass_kernel_spmd(nc, in_maps, core_ids=list(range(NCORES)))
    outs = [
        r["out"].astype(np.float32).reshape(BPC, C, H, W) for r in res.results
    ]
    return np.concatenate(outs, axis=0)


# revision 14
# speedup vs baseline: 1.2271x; 1.2271x over previous
"""Trainium2 Bass kernel for nn_HardMemory (retrieval_knn).

For each spatial token (B*H*W tokens, C=128 channels), find the memory row
(of M=512) with max cosine similarity and replace the token's channel vector
with that raw memory row.

Algebraic simplification: argmax_m cos(x, mem_m) = argmax_m (x . mem_n_m)
where mem_n is the l2-normalized memory -- normalizing x is a positive
per-token scale and cannot change the argmax, so it is skipped.

Scores (PSUM fp32, beta-scaled): s = xh.mh16 (one fp16 matmul) plus two fp8
DoubleRow matmuls carrying the precision cross-terms at half cost:
  DR-A: (xl1, xl2).(mhA, mhB)   ~= xl.mh      (xl = fp16 residual of x,
        split in two e4m3 terms pre-scaled by 2^16; mh slots down-scaled)
  DR-B: (xh8a, xh8b).(mlA, mlB) ~= xh.ml      (ml = fp16 residual of the
        beta-scaled memory; xh split in two e4m3 terms)
Each DoubleRow sums TWO independent 128-deep products at 0.5 cycles/row, so
per-tile score cost is 512+256+256 = 1024 PE cycles vs 1536 for the 3-term
fp16 scheme. Dropped terms are O(2^-15) of score scale; measured ~10 argmax
flips over 131072 tokens, well under the 2e-2 rel-err gate.

One-hot: exp(s - max) on ACT (bias = negated reduce_max, scale=1), written
as e4m3 bytes at stride 2 into a SHARED fp16 tile: tile j of the group owns
byte plane j. One fp16 128x128 transpose then moves BOTH tiles' one-hot
bytes at once (one-hot bytes are <= 0x38 so the fp16 lanes are always
finite positive and the identity-matmul transpose is exact). This halves
both the PE transpose cost and the PSUM->SBUF copy volume vs per-tile fp8
transposes.

The gather reconstructs memory rows from a 2-term fp8 (e4m3 hi+lo) split
via fp8 DoubleRow one-hot matmuls reading stride-2 byte views of the
transposed pack: per tile 4 DR matmuls (2 chunk-pairs x hi/lo) at 0.5
cycles/row. Output fp32 PSUM -> fp16 SBUF copy -> DMA, stored fp16 and
upcast on the host (fp8 recon err ~1e-3 rel, far under the gate).

Engine balance per group (2 tiles, 256 tokens), ~64 groups per core:
  PE  : scores 853ns + transposes 213ns + gather 213ns  = 1280ns
  DVE : batched reduce_max (negate)                     = 1192ns (+ a few
        out-copies)
  ACT : 2 exps                                          = 1224ns
  Pool: packed ohT copy 806ns + most out-copies ~450ns  = ~1210ns
All four engines sit at 76-82us of ~87us span.

Sharding: data-parallel over batch, 4 batches per core, memory replicated.
Input DMAs are sliced and spread across groups so the 625ns HWDGE
descriptor setups never serialize against compute.
"""

import numpy as np

import concourse.bass as bass
import concourse.mybir as mybir
from concourse.tile import TileContext
from concourse.bass_utils import run_bass_kernel_spmd

F32 = mybir.dt.float32
F16 = mybir.dt.float16
F8 = mybir.dt.float8e4
AF = mybir.ActivationFunctionType

B, C, H, W = 32, 128, 64, 64
N = H * W              # 4096 tokens per batch
M = 512                # memory rows
NCORES = 8
BPC = B // NCORES      # batches per core
TOK = BPC * N          # tokens per core
TILE = 128             # tokens per tile
GRP = 2                # tiles per PSUM score group
LOAD = 4096            # tokens per input DMA region (one full batch image)
STORE = 512            # tokens per output DMA chunk (2 groups)
KCH = M // TILE        # 4 gather chunks
SIG = 65536.0          # xl pre-scale for the fp8 split (exact power of 2)


def _build():
    nc = bass.Bass(trn_type="TRN2")

    xh_in = nc.dram_tensor("xh", [BPC, C, N], F16, kind="ExternalInput")
    # fp8 slots: 0=xl1 1=xl2 (DR-A lhsT pair), 2=xh8a 3=xh8b (DR-B pair)
    x8_in = nc.dram_tensor("x8", [BPC, C, 4, N], F8, kind="ExternalInput")
    mh_in = nc.dram_tensor("mh16", [C, M], F16, kind="ExternalInput")
    # fp8 slots: 0=mhA 1=mhB (DR-A rhs pair), 2=mlA 3=mlB (DR-B rhs pair)
    m8_in = nc.dram_tensor("m8", [C, 4, M], F8, kind="ExternalInput")
    # raw memory rows fp8 hi/lo [TILE, KCH, 2, C] packed
    g8_in = nc.dram_tensor("g8", [TILE, KCH * 2 * C], F8, kind="ExternalInput")
    idu_in = nc.dram_tensor("idu", [TILE, TILE], F16, kind="ExternalInput")
    out_d = nc.dram_tensor("out", [BPC, C, N], F16, kind="ExternalOutput")

    with TileContext(nc) as tc:
        with (
            tc.tile_pool(name="const", bufs=1) as cpool,
            tc.tile_pool(name="xin", bufs=4) as xpool,
            tc.tile_pool(name="oh", bufs=6) as ohpool,
            tc.tile_pool(name="oht", bufs=4) as ohtpool,
            tc.tile_pool(name="osb", bufs=4) as opool,
            tc.tile_pool(name="small", bufs=12) as spool,
            tc.tile_pool(name="ps_s", bufs=4, space="PSUM") as ps_s,
            tc.tile_pool(name="ps_t", bufs=2, space="PSUM") as ps_t,
            tc.tile_pool(name="ps_o", bufs=2, space="PSUM") as ps_o,
        ):
            n_groups = TOK // (TILE * GRP)
            grp_per_batch = N // (TILE * GRP)
            loaded = {}

            def load_slice(b, s0, s1):
                nb = 8 if s1 - s0 == 1024 else 1
                xh_sb = xpool.tile(
                    [C, s1 - s0], F16, tag=f"xh{s1 - s0}", bufs=nb
                )
                nc.sync.dma_start(out=xh_sb, in_=xh_in[b, :, s0:s1])
                x8_sb = xpool.tile(
                    [C, 4, s1 - s0], F8, tag=f"x8{s1 - s0}", bufs=nb
                )
                nc.sync.dma_start(out=x8_sb, in_=x8_in[b, :, :, s0:s1])
                loaded.setdefault(b, []).append((s0, s1, xh_sb, x8_sb))

            def xslice(b, o, size):
                for s0, s1, xh_sb, x8_sb in loaded[b]:
                    if s0 <= o and o + size <= s1:
                        return (xh_sb[:, o - s0 : o - s0 + size],
                                x8_sb[:, :, o - s0 : o - s0 + size])
                raise AssertionError((b, o, size))

            # Prefetch schedule: batches 1..3 load one 1024-token slice per
            # group, starting 12 groups before the batch is needed.
            load_plan = {}
            for b in range(1, BPC):
                for si in range(4):
                    load_plan.setdefault(
                        b * grp_per_batch - 12 + 2 * si, []
                    ).append((b, si * 1024, (si + 1) * 1024))

            mh16 = cpool.tile([C, M], F16)
            nc.sync.dma_start(out=mh16, in_=mh_in[:])
            m8 = cpool.tile([C, 4, M], F8)
            nc.sync.dma_start(out=m8, in_=m8_in[:])
            # batch 0: small leading slices so PE starts early
            for s0, s1 in ((0, 256), (256, 1024), (1024, 2048), (2048, 3072),
                           (3072, N)):
                load_slice(0, s0, s1)
            g8 = cpool.tile([TILE, KCH * 2 * C], F8)
            nc.sync.dma_start(out=g8, in_=g8_in[:])
            g8v = g8.rearrange("p (k h c) -> p k h c", k=KCH, h=2)
            idu = cpool.tile([TILE, TILE], F16)
            nc.sync.dma_start(out=idu, in_=idu_in[:])

            DR = mybir.MatmulPerfMode.DoubleRow

            def head(g):
                """Score matmuls for group g + per-tile negated max.
                Each tile gets its OWN 1-bank PSUM tile and reduce so the
                buffer-release semaphore fires per tile: head(g+2) tile j
                only waits on exp(g) tile j, keeping the PSUM-recycle cycle
                well under 2x the engine-busy period."""
                gtok0 = g * TILE * GRP
                b = gtok0 // N
                for lb, s0, s1 in load_plan.get(g, ()):
                    load_slice(lb, s0, s1)
                psts = []
                nbmxs = []
                for j in range(GRP):
                    tok0 = gtok0 + j * TILE
                    o = tok0 % LOAD
                    xht, x8t = xslice(b, o, TILE)
                    ps = ps_s.tile([TILE, M], F32, tag="pst")
                    nc.tensor.matmul(out=ps, lhsT=xht, rhs=mh16,
                                     start=True, stop=False)
                    nc.tensor.matmul(out=ps, lhsT=x8t[:, 0:2, :],
                                     rhs=m8[:, 0:2, :],
                                     start=False, stop=False, perf_mode=DR)
                    nc.tensor.matmul(out=ps, lhsT=x8t[:, 2:4, :],
                                     rhs=m8[:, 2:4, :],
                                     start=False, stop=True, perf_mode=DR)
                    nbmx = spool.tile([TILE, 1], F32, tag="nbmx")
                    nc.vector.reduce_max(
                        out=nbmx, in_=ps, axis=mybir.AxisListType.X,
                        negate=True,
                    )
                    psts.append(ps)
                    nbmxs.append(nbmx)
                return psts, nbmxs

            def exp_stage(g, psts, nbmxs):
                """Exp one-hot for group g (1 group late): both tiles write
                fp8 bytes into one shared fp16-typed pack tile (tile j owns
                byte plane j)."""
                ohp = ohpool.tile([TILE, M], F16)
                oh8 = ohp.bitcast(F8).rearrange("p (m two) -> p m two", two=2)
                for j in range(GRP):
                    nc.scalar.activation(
                        out=oh8[:, :, j], in_=psts[j], func=AF.Exp,
                        bias=nbmxs[j], scale=1.0,
                    )
                return ohp

            psT_cur = [None]

            def tr_stage(g, ohp):
                """4 fp16 transposes move both byte planes at once (2 groups
                late). Pairs of groups share one 1-bank PSUM tile."""
                if g % 2 == 0:
                    psT_tile = ps_t.tile([TILE, 2, KCH, TILE], F16, tag="psT")
                    psT_cur[0] = psT_tile
                psT = psT_cur[0]
                for k in range(KCH):
                    nc.tensor.transpose(
                        out=psT[:, g % 2, k, :],
                        in_=ohp[:, k * TILE : (k + 1) * TILE],
                        identity=idu,
                    )
                return psT if g % 2 == 1 else None

            def copy_stage(psT, p):
                """One batched copy (DVE 2x fp16 mode; every 8th pair on ACT
                to shave the DVE bottleneck) moves a pair's transposed
                one-hots to SBUF, one iteration after the last transpose.
                (GPSIMD cannot access PSUM, so DVE/ACT carry all copies.)"""
                ohT = ohtpool.tile([TILE, 2, KCH, TILE], F16)
                if p % 8 == 3:
                    nc.scalar.activation(out=ohT, in_=psT, func=AF.Copy)
                else:
                    nc.vector.tensor_copy(ohT, psT)
                return ohT

            po_cur = [None]

            def gather_stage(g, ohT, half):
                """fp8 DoubleRow gather (3 groups late): per tile 4 DR
                matmuls (chunk-pair x hi/lo). Pairs of groups share one
                [C, 512] fp32 PSUM bank."""
                if g % 2 == 0:
                    po_tile = ps_o.tile([C, STORE], F32, tag="po")
                    po_cur[0] = po_tile
                po = po_cur[0]
                ohT8 = ohT.bitcast(F8).rearrange(
                    "p h k (t two) -> p h k t two", two=2
                )
                for j in range(GRP):
                    col0 = (g % 2) * GRP * TILE + j * TILE
                    n_mm = 0
                    for pr in range(KCH // 2):
                        for h in range(2):
                            nc.tensor.matmul(
                                out=po[:, col0 : col0 + TILE],
                                lhsT=g8v[:, 2 * pr : 2 * pr + 2, h, :],
                                rhs=ohT8[:, half, 2 * pr : 2 * pr + 2, :, j],
                                start=(n_mm == 0), stop=(n_mm == 2 * KCH - 1),
                                perf_mode=DR,
                            )
                            n_mm += 1
                return po if g % 2 == 1 else None

            def tail(g, po):
                """Batched out-copy on ACT (5 groups late): [C, 512] fp32
                PSUM -> fp16 SBUF, then one DMA per STORE tokens."""
                gtok0 = g * TILE * GRP
                ob = opool.tile([C, STORE], F16, tag="ob")
                nc.scalar.activation(out=ob, in_=po, func=AF.Copy)
                b, n0 = divmod(gtok0 + GRP * TILE - STORE, N)
                nc.sync.dma_start(out=out_d[b, :, n0 : n0 + STORE], in_=ob)

            # Software pipeline, one iteration per score group g:
            #   PE : tr(g-2) | gather(g-5) | head(g)     (ready work first;
            #        head's buf wait is the in-order SEQ block point)
            #   DVE: ohT copy (pair (g-4)//2) | reduce(g)
            #   ACT: exps(g-1) | tail copy(g-6)
            # The head->reduce->exp->head PSUM recycle is the critical cycle;
            # per-TILE subtile deps (exp tile j releases the score buf slice
            # for head(g+2) tile j) keep it under 2x the engine-busy period.
            p_head = None
            exps = {}
            psTs = {}
            ohts = {}
            pos = {}
            for g in range(n_groups + 10):
                # PE program order: head(g) | tr(g-2) | gather(g-6) -- oldest
                # dependencies last, so no stage's wait blocks a later
                # stage whose inputs are already ready. The ohT copy a PE
                # gather consumes is 2 iterations old; the exp slice a head
                # matmul waits on (score-buf recycle) is the critical cycle.
                g4 = g - 4
                if g4 >= 0 and g4 % 2 == 0 and (g4 // 2) in psTs:
                    ohts[g4 // 2] = copy_stage(psTs.pop(g4 // 2), g4 // 2)
                nxt_head = head(g) if g < n_groups else None
                if g - 2 >= 0 and (g - 2) in exps:
                    psT = tr_stage(g - 2, exps.pop(g - 2))
                    if psT is not None:
                        psTs[(g - 2) // 2] = psT
                g6 = g - 6
                if g6 >= 0 and (g6 // 2) in ohts:
                    po = gather_stage(g6, ohts[g6 // 2], half=g6 % 2)
                    if po is not None:
                        pos[g6 // 2] = po
                        del ohts[g6 // 2]
                if p_head is not None:
                    exps[g - 1] = exp_stage(g - 1, *p_head)
                g8 = g - 8
                if g8 >= 0 and g8 % 2 == 1 and (g8 // 2) in pos:
                    tail(g8, pos.pop(g8 // 2))
                p_head = nxt_head

    _legalize_waits(nc)
    nc.finalize()
    return nc


def _legalize_waits(nc):
    """This container's walrus accepts only ONE sync wait per engine
    instruction (setupSyncWait: 'Too many sync wait commands'). Tile emits
    multi-wait instructions (and an 11-wait tail drain). Split: keep one
    wait on the instruction, hoist the rest onto single-wait Drain ops
    inserted just before it on the same engine (engine order preserved =>
    semantics preserved). DMA copies are left alone (ring descriptors
    accept multiple waits)."""
    n_split = 0
    for f in nc.m.functions:
        for b in f.blocks:
            out = []
            for inst in b.instructions:
                si = inst.sync_info
                if si is not None and len(si.on_wait) > 1:
                    waits = list(si.on_wait)
                    for j, w in enumerate(waits[:-1]):
                        out.append(
                            mybir.InstDrain(
                                name=f"{inst.name}-w{j}",
                                engine=inst.engine,
                                ins=[],
                                outs=[],
                                sync_info=mybir.SyncInfo(
                                    on_wait=[w], on_update=[]
                                ),
                            )
                        )
                    inst.sync_info = mybir.SyncInfo(
                        on_wait=[waits[-1]], on_update=list(si.on_update)
                    )
                    n_split += 1
                out.append(inst)
            b.instructions = out
    return n_split


_NC = None


def _get_nc():
    global _NC
    if _NC is None:
        _NC = _build()
    return _NC


def _host_prep(x, memory):
    import ml_dtypes
    f8 = ml_dtypes.float8_e4m3

    memn = memory / np.maximum(
        np.sqrt((memory * memory).sum(axis=1, keepdims=True)), 1e-12
    )
    # Scale the normalized memory by BETA so PSUM holds beta*s directly
    # (exp sharpness K_eff = beta ~ 1e5). Cap keeps fp16 mh finite.
    beta = min(1e5, 55000.0 / max(float(np.abs(memn).max()), 1e-6))
    mnt = np.ascontiguousarray(memn.T).astype(np.float32) * beta   # [C, M]
    mh = mnt.astype(np.float16)
    ml = (mnt - mh.astype(np.float32)).astype(np.float32)

    # DR rhs slot tables (all exact power-of-2 shifts of fp8 encodings)
    mhf = mh.astype(np.float32)
    m8 = np.zeros((C, 4, M), dtype=f8)
    m8[:, 0, :] = (mhf / SIG).astype(f8)             # mhA
    m8[:, 1, :] = (mhf / (SIG * 16.0)).astype(f8)    # mhB
    m8[:, 2, :] = ml.astype(f8)                      # mlA
    m8[:, 3, :] = (ml / 16.0).astype(f8)             # mlB

    xh = x.astype(np.float16)
    xl = (x - xh.astype(np.float32)).astype(np.float32)
    xl1 = (xl * SIG).astype(f8)
    xl2 = ((xl * SIG - xl1.astype(np.float32)) * 16.0).astype(f8)
    xh8a = xh.astype(np.float32).astype(f8)
    xh8b = ((xh.astype(np.float32) - xh8a.astype(np.float32)) * 16.0).astype(f8)
    x8 = np.stack([xl1, xl2, xh8a, xh8b], axis=2)    # [B, C, 4, HW...]

    gh8 = memory.astype(f8)
    gl8 = (memory - gh8.astype(np.float32)).astype(f8)
    g8 = np.zeros((TILE, KCH * 2 * C), dtype=f8)
    for k in range(KCH):
        base = k * 2 * C
        g8[:, base : base + C] = gh8[k * TILE : (k + 1) * TILE, :]
        g8[:, base + C : base + 2 * C] = gl8[k * TILE : (k + 1) * TILE, :]

    return xh, x8, mh, m8, g8


def kernel(x, memory):
    x = np.asarray(x, dtype=np.float32)
    memory = np.asarray(memory, dtype=np.float32)
    nc = _get_nc()
    xf = x.reshape(B, C, N)
    xh, x8, mh16, m8, g8 = _host_prep(xf, memory)
    idu = np.eye(TILE, dtype=np.float16)

    in_maps = []
    for c in range(NCORES):
        in_maps.append({
            "xh": np.ascontiguousarray(xh[c * BPC : (c + 1) * BPC]),
            "x8": np.ascontiguousarray(x8[c * BPC : (c + 1) * BPC]),
            "mh16": mh16, "m8": m8, "g8": g8, "idu": idu,
        })

    res = run_bass_kernel_spmd(nc, in_maps, core_ids=list(range(NCORES)))
    outs = [
        r["out"].astype(np.float32).reshape(BPC, C, H, W) for r in res.results
    ]
    return np.concatenate(outs, axis=0)


# revision 16
# speedup vs baseline: 1.2279x; 1.0007x over previous
"""Trainium2 Bass kernel for nn_HardMemory (retrieval_knn).

For each spatial token (B*H*W tokens, C=128 channels), find the memory row
(of M=512) with max cosine similarity and replace the token's channel vector
with that raw memory row.

Algebraic simplification: argmax_m cos(x, mem_m) = argmax_m (x . mem_n_m)
where mem_n is the l2-normalized memory -- normalizing x is a positive
per-token scale and cannot change the argmax, so it is skipped.

Scores (PSUM fp32, beta-scaled): s = xh.mh16 (one fp16 matmul) plus two fp8
DoubleRow matmuls carrying the precision cross-terms at half cost:
  DR-A: (xl1, xl2).(mhA, mhB)   ~= xl.mh      (xl = fp16 residual of x,
        split in two e4m3 terms pre-scaled by 2^16; mh slots down-scaled)
  DR-B: (xh8a, xh8b).(mlA, mlB) ~= xh.ml      (ml = fp16 residual of the
        beta-scaled memory; xh split in two e4m3 terms)
Each DoubleRow sums TWO independent 128-deep products at 0.5 cycles/row, so
per-tile score cost is 512+256+256 = 1024 PE cycles vs 1536 for the 3-term
fp16 scheme. Dropped terms are O(2^-15) of score scale; measured ~10 argmax
flips over 131072 tokens, well under the 2e-2 rel-err gate.

One-hot: exp(s - max) on ACT (bias = negated reduce_max, scale=1), written
as e4m3 bytes at stride 2 into a SHARED fp16 tile: tile j of the group owns
byte plane j. One fp16 128x128 transpose then moves BOTH tiles' one-hot
bytes at once (one-hot bytes are <= 0x38 so the fp16 lanes are always
finite positive and the identity-matmul transpose is exact). This halves
both the PE transpose cost and the PSUM->SBUF copy volume vs per-tile fp8
transposes.

The gather reconstructs memory rows from a 2-term fp8 (e4m3 hi+lo) split
via fp8 DoubleRow one-hot matmuls reading stride-2 byte views of the
transposed pack: per tile 4 DR matmuls (2 chunk-pairs x hi/lo) at 0.5
cycles/row. Output fp32 PSUM -> fp16 SBUF copy -> DMA, stored fp16 and
upcast on the host (fp8 recon err ~1e-3 rel, far under the gate).

Engine balance per group (2 tiles, 256 tokens), ~64 groups per core:
  PE  : scores 853ns + transposes 213ns + gather 213ns  = 1280ns
  DVE : batched reduce_max (negate)                     = 1192ns (+ a few
        out-copies)
  ACT : 2 exps                                          = 1224ns
  Pool: packed ohT copy 806ns + most out-copies ~450ns  = ~1210ns
All four engines sit at 76-82us of ~87us span.

Sharding: data-parallel over batch, 4 batches per core, memory replicated.
Input DMAs are sliced and spread across groups so the 625ns HWDGE
descriptor setups never serialize against compute.
"""

import numpy as np

import concourse.bass as bass
import concourse.mybir as mybir
from concourse.tile import TileContext
from concourse.bass_utils import run_bass_kernel_spmd

F32 = mybir.dt.float32
F16 = mybir.dt.float16
F8 = mybir.dt.float8e4
AF = mybir.ActivationFunctionType

B, C, H, W = 32, 128, 64, 64
N = H * W              # 4096 tokens per batch
M = 512                # memory rows
NCORES = 8
BPC = B // NCORES      # batches per core
TOK = BPC * N          # tokens per core
TILE = 128             # tokens per tile
GRP = 2                # tiles per PSUM score group
LOAD = 4096            # tokens per input DMA region (one full batch image)
STORE = 512            # tokens per output DMA chunk (2 groups)
KCH = M // TILE        # 4 gather chunks
SIG = 65536.0          # xl pre-scale for the fp8 split (exact power of 2)


def _build():
    nc = bass.Bass(trn_type="TRN2")

    xh_in = nc.dram_tensor("xh", [BPC, C, N], F16, kind="ExternalInput")
    # fp8 slots: 0=xl1 1=xl2 (DR-A lhsT pair), 2=xh8a 3=xh8b (DR-B pair)
    x8_in = nc.dram_tensor("x8", [BPC, C, 4, N], F8, kind="ExternalInput")
    mh_in = nc.dram_tensor("mh16", [C, M], F16, kind="ExternalInput")
    # fp8 slots: 0=mhA 1=mhB (DR-A rhs pair), 2=mlA 3=mlB (DR-B rhs pair)
    m8_in = nc.dram_tensor("m8", [C, 4, M], F8, kind="ExternalInput")
    # raw memory rows fp8 hi/lo [TILE, KCH, 2, C] packed
    g8_in = nc.dram_tensor("g8", [TILE, KCH * 2 * C], F8, kind="ExternalInput")
    idu_in = nc.dram_tensor("idu", [TILE, TILE], F16, kind="ExternalInput")
    out_d = nc.dram_tensor("out", [BPC, C, N], F16, kind="ExternalOutput")

    with TileContext(nc) as tc:
        with (
            tc.tile_pool(name="const", bufs=1) as cpool,
            tc.tile_pool(name="xin", bufs=4) as xpool,
            tc.tile_pool(name="oh", bufs=6) as ohpool,
            tc.tile_pool(name="oht", bufs=4) as ohtpool,
            tc.tile_pool(name="osb", bufs=4) as opool,
            tc.tile_pool(name="small", bufs=12) as spool,
            tc.tile_pool(name="ps_s", bufs=4, space="PSUM") as ps_s,
            tc.tile_pool(name="ps_t", bufs=2, space="PSUM") as ps_t,
            tc.tile_pool(name="ps_o", bufs=2, space="PSUM") as ps_o,
        ):
            n_groups = TOK // (TILE * GRP)
            grp_per_batch = N // (TILE * GRP)
            loaded = {}

            def load_slice(b, s0, s1):
                nb = 8 if s1 - s0 == 1024 else 1
                xh_sb = xpool.tile(
                    [C, s1 - s0], F16, tag=f"xh{s1 - s0}", bufs=nb
                )
                nc.sync.dma_start(out=xh_sb, in_=xh_in[b, :, s0:s1])
                x8_sb = xpool.tile(
                    [C, 4, s1 - s0], F8, tag=f"x8{s1 - s0}", bufs=nb
                )
                nc.sync.dma_start(out=x8_sb, in_=x8_in[b, :, :, s0:s1])
                loaded.setdefault(b, []).append((s0, s1, xh_sb, x8_sb))

            def xslice(b, o, size):
                for s0, s1, xh_sb, x8_sb in loaded[b]:
                    if s0 <= o and o + size <= s1:
                        return (xh_sb[:, o - s0 : o - s0 + size],
                                x8_sb[:, :, o - s0 : o - s0 + size])
                raise AssertionError((b, o, size))

            # Prefetch schedule: batches 1..3 load one 1024-token slice per
            # group, starting 12 groups before the batch is needed.
            load_plan = {}
            for b in range(1, BPC):
                for si in range(4):
                    load_plan.setdefault(
                        b * grp_per_batch - 12 + 2 * si, []
                    ).append((b, si * 1024, (si + 1) * 1024))

            mh16 = cpool.tile([C, M], F16)
            nc.sync.dma_start(out=mh16, in_=mh_in[:])
            m8 = cpool.tile([C, 4, M], F8)
            nc.sync.dma_start(out=m8, in_=m8_in[:])
            # batch 0: small leading slices so PE starts early
            for s0, s1 in ((0, 256), (256, 1024), (1024, 2048), (2048, 3072),
                           (3072, N)):
                load_slice(0, s0, s1)
            g8 = cpool.tile([TILE, KCH * 2 * C], F8)
            nc.sync.dma_start(out=g8, in_=g8_in[:])
            g8v = g8.rearrange("p (k h c) -> p k h c", k=KCH, h=2)
            idu = cpool.tile([TILE, TILE], F16)
            nc.sync.dma_start(out=idu, in_=idu_in[:])

            DR = mybir.MatmulPerfMode.DoubleRow

            def head(g):
                """Score matmuls for group g + per-tile negated max.
                Each tile gets its OWN 1-bank PSUM tile and reduce so the
                buffer-release semaphore fires per tile: head(g+2) tile j
                only waits on exp(g) tile j, keeping the PSUM-recycle cycle
                well under 2x the engine-busy period."""
                gtok0 = g * TILE * GRP
                b = gtok0 // N
                for lb, s0, s1 in load_plan.get(g, ()):
                    load_slice(lb, s0, s1)
                psts = []
                nbmxs = []
                for j in range(GRP):
                    tok0 = gtok0 + j * TILE
                    o = tok0 % LOAD
                    xht, x8t = xslice(b, o, TILE)
                    ps = ps_s.tile([TILE, M], F32, tag="pst")
                    nc.tensor.matmul(out=ps, lhsT=xht, rhs=mh16,
                                     start=True, stop=False)
                    nc.tensor.matmul(out=ps, lhsT=x8t[:, 0:2, :],
                                     rhs=m8[:, 0:2, :],
                                     start=False, stop=False, perf_mode=DR)
                    nc.tensor.matmul(out=ps, lhsT=x8t[:, 2:4, :],
                                     rhs=m8[:, 2:4, :],
                                     start=False, stop=True, perf_mode=DR)
                    nbmx = spool.tile([TILE, 1], F32, tag="nbmx")
                    nc.vector.reduce_max(
                        out=nbmx, in_=ps, axis=mybir.AxisListType.X,
                        negate=True,
                    )
                    psts.append(ps)
                    nbmxs.append(nbmx)
                return psts, nbmxs

            def exp_stage(g, psts, nbmxs):
                """Exp one-hot for group g (1 group late): both tiles write
                fp8 bytes into one shared fp16-typed pack tile (tile j owns
                byte plane j)."""
                ohp = ohpool.tile([TILE, M], F16)
                oh8 = ohp.bitcast(F8).rearrange("p (m two) -> p m two", two=2)
                for j in range(GRP):
                    nc.scalar.activation(
                        out=oh8[:, :, j], in_=psts[j], func=AF.Exp,
                        bias=nbmxs[j], scale=1.0,
                    )
                return ohp

            def tr_stage(g, ohp):
                """4 fp16 transposes move both byte planes at once (2 groups
                late) into a per-group 1-bank PSUM tile."""
                psT = ps_t.tile([TILE, KCH, TILE], F16, tag="psT")
                for k in range(KCH):
                    nc.tensor.transpose(
                        out=psT[:, k, :],
                        in_=ohp[:, k * TILE : (k + 1) * TILE],
                        identity=idu,
                    )
                return psT

            def copy_stage(psT, g):
                """Per-group copy (DVE 2x fp16 mode; every 8th on ACT to
                shave the DVE bottleneck) moves the transposed one-hots to
                SBUF, one iteration after the transposes. (GPSIMD cannot
                access PSUM, so DVE/ACT carry all copies.)"""
                ohT = ohtpool.tile([TILE, KCH, TILE], F16)
                if g % 8 == 3:
                    nc.scalar.activation(out=ohT, in_=psT, func=AF.Copy)
                else:
                    nc.vector.tensor_copy(ohT, psT)
                return ohT

            po_cur = [None]

            def gather_stage(g, ohT, half):
                """fp8 DoubleRow gather (3 groups late): per tile 4 DR
                matmuls (chunk-pair x hi/lo). Pairs of groups share one
                [C, 512] fp32 PSUM bank."""
                if g % 2 == 0:
                    po_tile = ps_o.tile([C, STORE], F32, tag="po")
                    po_cur[0] = po_tile
                po = po_cur[0]
                ohT8 = ohT.bitcast(F8).rearrange(
                    "p k (t two) -> p k t two", two=2
                )
                for j in range(GRP):
                    col0 = (g % 2) * GRP * TILE + j * TILE
                    n_mm = 0
                    for pr in range(KCH // 2):
                        for h in range(2):
                            nc.tensor.matmul(
                                out=po[:, col0 : col0 + TILE],
                                lhsT=g8v[:, 2 * pr : 2 * pr + 2, h, :],
                                rhs=ohT8[:, 2 * pr : 2 * pr + 2, :, j],
                                start=(n_mm == 0), stop=(n_mm == 2 * KCH - 1),
                                perf_mode=DR,
                            )
                            n_mm += 1
                return po if g % 2 == 1 else None

            def tail(g, po):
                """Batched out-copy on ACT (5 groups late): [C, 512] fp32
                PSUM -> fp16 SBUF, then one DMA per STORE tokens."""
                gtok0 = g * TILE * GRP
                ob = opool.tile([C, STORE], F16, tag="ob")
                nc.scalar.activation(out=ob, in_=po, func=AF.Copy)
                b, n0 = divmod(gtok0 + GRP * TILE - STORE, N)
                nc.sync.dma_start(out=out_d[b, :, n0 : n0 + STORE], in_=ob)

            # Software pipeline, one iteration per score group g:
            #   PE : tr(g-2) | gather(g-5) | head(g)     (ready work first;
            #        head's buf wait is the in-order SEQ block point)
            #   DVE: ohT copy (pair (g-4)//2) | reduce(g)
            #   ACT: exps(g-1) | tail copy(g-6)
            # The head->reduce->exp->head PSUM recycle is the critical cycle;
            # per-TILE subtile deps (exp tile j releases the score buf slice
            # for head(g+2) tile j) keep it under 2x the engine-busy period.
            p_head = None
            exps = {}
            psTs = {}
            ohts = {}
            pos = {}
            for g in range(n_groups + 8):
                # Per-iteration stages: head(g) | exp(g-1) | tr(g-2) |
                # copy(g-3) | gather(g-4) | tail(g-6, per pair). Every
                # cross-engine dependency is >=1 iteration old; the score-buf
                # recycle (head(g+2) tile j <- exp(g) tile j) is the
                # critical cycle, kept short by per-tile PSUM tiles.
                g3 = g - 3
                if g3 >= 0 and g3 in psTs:
                    ohts[g3] = copy_stage(psTs.pop(g3), g3)
                nxt_head = head(g) if g < n_groups else None
                if g - 2 >= 0 and (g - 2) in exps:
                    psTs[g - 2] = tr_stage(g - 2, exps.pop(g - 2))
                g4 = g - 4
                if g4 >= 0 and g4 in ohts:
                    po = gather_stage(g4, ohts.pop(g4), half=g4 % 2)
                    if po is not None:
                        pos[g4 // 2] = po
                if p_head is not None:
                    exps[g - 1] = exp_stage(g - 1, *p_head)
                g6 = g - 6
                if g6 >= 0 and g6 % 2 == 1 and (g6 // 2) in pos:
                    tail(g6, pos.pop(g6 // 2))
                p_head = nxt_head

    _legalize_waits(nc)
    nc.finalize()
    return nc


def _legalize_waits(nc):
    """This container's walrus accepts only ONE sync wait per engine
    instruction (setupSyncWait: 'Too many sync wait commands'). Tile emits
    multi-wait instructions (and an 11-wait tail drain). Split: keep one
    wait on the instruction, hoist the rest onto single-wait Drain ops
    inserted just before it on the same engine (engine order preserved =>
    semantics preserved). DMA copies are left alone (ring descriptors
    accept multiple waits)."""
    n_split = 0
    for f in nc.m.functions:
        for b in f.blocks:
            out = []
            for inst in b.instructions:
                si = inst.sync_info
                if si is not None and len(si.on_wait) > 1:
                    waits = list(si.on_wait)
                    for j, w in enumerate(waits[:-1]):
                        out.append(
                            mybir.InstDrain(
                                name=f"{inst.name}-w{j}",
                                engine=inst.engine,
                                ins=[],
                                outs=[],
                                sync_info=mybir.SyncInfo(
                                    on_wait=[w], on_update=[]
                                ),
                            )
                        )
                    inst.sync_info = mybir.SyncInfo(
                        on_wait=[waits[-1]], on_update=list(si.on_update)
                    )
                    n_split += 1
                out.append(inst)
            b.instructions = out
    return n_split


_NC = None


def _get_nc():
    global _NC
    if _NC is None:
        _NC = _build()
    return _NC


def _host_prep(x, memory):
    import ml_dtypes
    f8 = ml_dtypes.float8_e4m3

    memn = memory / np.maximum(
        np.sqrt((memory * memory).sum(axis=1, keepdims=True)), 1e-12
    )
    # Scale the normalized memory by BETA so PSUM holds beta*s directly
    # (exp sharpness K_eff = beta ~ 1e5). Cap keeps fp16 mh finite.
    beta = min(1e5, 55000.0 / max(float(np.abs(memn).max()), 1e-6))
    mnt = np.ascontiguousarray(memn.T).astype(np.float32) * beta   # [C, M]
    mh = mnt.astype(np.float16)
    ml = (mnt - mh.astype(np.float32)).astype(np.float32)

    # DR rhs slot tables (all exact power-of-2 shifts of fp8 encodings)
    mhf = mh.astype(np.float32)
    m8 = np.zeros((C, 4, M), dtype=f8)
    m8[:, 0, :] = (mhf / SIG).astype(f8)             # mhA
    m8[:, 1, :] = (mhf / (SIG * 16.0)).astype(f8)    # mhB
    m8[:, 2, :] = ml.astype(f8)                      # mlA
    m8[:, 3, :] = (ml / 16.0).astype(f8)             # mlB

    xh = x.astype(np.float16)
    xl = (x - xh.astype(np.float32)).astype(np.float32)
    xl1 = (xl * SIG).astype(f8)
    xl2 = ((xl * SIG - xl1.astype(np.float32)) * 16.0).astype(f8)
    xh8a = xh.astype(np.float32).astype(f8)
    xh8b = ((xh.astype(np.float32) - xh8a.astype(np.float32)) * 16.0).astype(f8)
    x8 = np.stack([xl1, xl2, xh8a, xh8b], axis=2)    # [B, C, 4, HW...]

    gh8 = memory.astype(f8)
    gl8 = (memory - gh8.astype(np.float32)).astype(f8)
    g8 = np.zeros((TILE, KCH * 2 * C), dtype=f8)
    for k in range(KCH):
        base = k * 2 * C
        g8[:, base : base + C] = gh8[k * TILE : (k + 1) * TILE, :]
        g8[:, base + C : base + 2 * C] = gl8[k * TILE : (k + 1) * TILE, :]

    return xh, x8, mh, m8, g8


def kernel(x, memory):
    x = np.asarray(x, dtype=np.float32)
    memory = np.asarray(memory, dtype=np.float32)
    nc = _get_nc()
    xf = x.reshape(B, C, N)
    xh, x8, mh16, m8, g8 = _host_prep(xf, memory)
    idu = np.eye(TILE, dtype=np.float16)

    in_maps = []
    for c in range(NCORES):
        in_maps.append({
            "xh": np.ascontiguousarray(xh[c * BPC : (c + 1) * BPC]),
            "x8": np.ascontiguousarray(x8[c * BPC : (c + 1) * BPC]),
            "mh16": mh16, "m8": m8, "g8": g8, "idu": idu,
        })

    res = run_bass_kernel_spmd(nc, in_maps, core_ids=list(range(NCORES)))
    outs = [
        r["out"].astype(np.float32).reshape(BPC, C, H, W) for r in res.results
    ]
    return np.concatenate(outs, axis=0)


# revision 17
# speedup vs baseline: 1.2427x; 1.0121x over previous
"""Trainium2 Bass kernel for nn_HardMemory (retrieval_knn).

For each spatial token (B*H*W tokens, C=128 channels), find the memory row
(of M=512) with max cosine similarity and replace the token's channel vector
with that raw memory row.

Algebraic simplification: argmax_m cos(x, mem_m) = argmax_m (x . mem_n_m)
where mem_n is the l2-normalized memory -- normalizing x is a positive
per-token scale and cannot change the argmax, so it is skipped.

Scores (PSUM fp32, beta-scaled): s = xh.mh16 (one fp16 matmul) plus two fp8
DoubleRow matmuls carrying the precision cross-terms at half cost:
  DR-A: (xl1, xl2).(mhA, mhB)   ~= xl.mh      (xl = fp16 residual of x,
        split in two e4m3 terms pre-scaled by 2^16; mh slots down-scaled)
  DR-B: (xh8a, xh8b).(mlA, mlB) ~= xh.ml      (ml = fp16 residual of the
        beta-scaled memory; xh split in two e4m3 terms)
Each DoubleRow sums TWO independent 128-deep products at 0.5 cycles/row, so
per-tile score cost is 512+256+256 = 1024 PE cycles vs 1536 for the 3-term
fp16 scheme. Dropped terms are O(2^-15) of score scale; measured ~10 argmax
flips over 131072 tokens, well under the 2e-2 rel-err gate.

One-hot: exp(s - max) on ACT (bias = negated reduce_max, scale=1), written
as e4m3 bytes at stride 2 into a SHARED fp16 tile: tile j of the group owns
byte plane j. One fp16 128x128 transpose then moves BOTH tiles' one-hot
bytes at once (one-hot bytes are <= 0x38 so the fp16 lanes are always
finite positive and the identity-matmul transpose is exact). This halves
both the PE transpose cost and the PSUM->SBUF copy volume vs per-tile fp8
transposes.

The gather reconstructs memory rows from a 2-term fp8 (e4m3 hi+lo) split
via fp8 DoubleRow one-hot matmuls reading stride-2 byte views of the
transposed pack: per tile 4 DR matmuls (2 chunk-pairs x hi/lo) at 0.5
cycles/row. Output fp32 PSUM -> fp16 SBUF copy -> DMA, stored fp16 and
upcast on the host (fp8 recon err ~1e-3 rel, far under the gate).

Engine balance per group (2 tiles, 256 tokens), ~64 groups per core:
  PE  : scores 853ns + transposes 213ns + gather 213ns  = 1280ns
  DVE : batched reduce_max (negate)                     = 1192ns (+ a few
        out-copies)
  ACT : 2 exps                                          = 1224ns
  Pool: packed ohT copy 806ns + most out-copies ~450ns  = ~1210ns
All four engines sit at 76-82us of ~87us span.

Sharding: data-parallel over batch, 4 batches per core, memory replicated.
Input DMAs are sliced and spread across groups so the 625ns HWDGE
descriptor setups never serialize against compute.
"""

import numpy as np

import concourse.bass as bass
import concourse.mybir as mybir
from concourse.tile import TileContext
from concourse.bass_utils import run_bass_kernel_spmd

F32 = mybir.dt.float32
F16 = mybir.dt.float16
F8 = mybir.dt.float8e4
AF = mybir.ActivationFunctionType

B, C, H, W = 32, 128, 64, 64
N = H * W              # 4096 tokens per batch
M = 512                # memory rows
NCORES = 8
BPC = B // NCORES      # batches per core
TOK = BPC * N          # tokens per core
TILE = 128             # tokens per tile
GRP = 2                # tiles per PSUM score group
LOAD = 4096            # tokens per input DMA region (one full batch image)
STORE = 512            # tokens per output DMA chunk (2 groups)
KCH = M // TILE        # 4 gather chunks
SIG = 65536.0          # xl pre-scale for the fp8 split (exact power of 2)


def _build():
    nc = bass.Bass(trn_type="TRN2")

    xh_in = nc.dram_tensor("xh", [BPC, C, N], F16, kind="ExternalInput")
    # fp8 slots: 0=xl1 1=xl2 (DR-A lhsT pair), 2=xh8a 3=xh8b (DR-B pair)
    x8_in = nc.dram_tensor("x8", [BPC, C, 4, N], F8, kind="ExternalInput")
    mh_in = nc.dram_tensor("mh16", [C, M], F16, kind="ExternalInput")
    # fp8 slots: 0=mhA 1=mhB (DR-A rhs pair), 2=mlA 3=mlB (DR-B rhs pair)
    m8_in = nc.dram_tensor("m8", [C, 4, M], F8, kind="ExternalInput")
    # raw memory rows fp8 hi/lo [TILE, KCH, 2, C] packed
    g8_in = nc.dram_tensor("g8", [TILE, KCH * 2 * C], F8, kind="ExternalInput")
    idu_in = nc.dram_tensor("idu", [TILE, TILE], F16, kind="ExternalInput")
    out_d = nc.dram_tensor("out", [BPC, C, N], F16, kind="ExternalOutput")

    with TileContext(nc) as tc:
        with (
            tc.tile_pool(name="const", bufs=1) as cpool,
            tc.tile_pool(name="xin", bufs=4) as xpool,
            tc.tile_pool(name="oh", bufs=6) as ohpool,
            tc.tile_pool(name="oht", bufs=4) as ohtpool,
            tc.tile_pool(name="osb", bufs=4) as opool,
            tc.tile_pool(name="small", bufs=12) as spool,
            tc.tile_pool(name="ps_s", bufs=4, space="PSUM") as ps_s,
            tc.tile_pool(name="ps_t", bufs=2, space="PSUM") as ps_t,
            tc.tile_pool(name="ps_o", bufs=2, space="PSUM") as ps_o,
        ):
            n_groups = TOK // (TILE * GRP)
            grp_per_batch = N // (TILE * GRP)
            loaded = {}

            def load_slice(b, s0, s1):
                nb = 8 if s1 - s0 == 1024 else 1
                xh_sb = xpool.tile(
                    [C, s1 - s0], F16, tag=f"xh{s1 - s0}", bufs=nb
                )
                nc.sync.dma_start(out=xh_sb, in_=xh_in[b, :, s0:s1])
                x8_sb = xpool.tile(
                    [C, 4, s1 - s0], F8, tag=f"x8{s1 - s0}", bufs=nb
                )
                nc.sync.dma_start(out=x8_sb, in_=x8_in[b, :, :, s0:s1])
                loaded.setdefault(b, []).append((s0, s1, xh_sb, x8_sb))

            def xslice(b, o, size):
                for s0, s1, xh_sb, x8_sb in loaded[b]:
                    if s0 <= o and o + size <= s1:
                        return (xh_sb[:, o - s0 : o - s0 + size],
                                x8_sb[:, :, o - s0 : o - s0 + size])
                raise AssertionError((b, o, size))

            # Prefetch schedule: batches 1..3 load one 1024-token slice per
            # group, starting 12 groups before the batch is needed.
            load_plan = {}
            for b in range(1, BPC):
                for si in range(4):
                    load_plan.setdefault(
                        b * grp_per_batch - 12 + 2 * si, []
                    ).append((b, si * 1024, (si + 1) * 1024))

            mh16 = cpool.tile([C, M], F16)
            nc.sync.dma_start(out=mh16, in_=mh_in[:])
            m8 = cpool.tile([C, 4, M], F8)
            nc.sync.dma_start(out=m8, in_=m8_in[:])
            # batch 0: small leading slices so PE starts early
            for s0, s1 in ((0, 256), (256, 1024), (1024, 2048), (2048, 3072),
                           (3072, N)):
                load_slice(0, s0, s1)
            g8 = cpool.tile([TILE, KCH * 2 * C], F8)
            nc.sync.dma_start(out=g8, in_=g8_in[:])
            g8v = g8.rearrange("p (k h c) -> p k h c", k=KCH, h=2)
            idu = cpool.tile([TILE, TILE], F16)
            nc.sync.dma_start(out=idu, in_=idu_in[:])

            DR = mybir.MatmulPerfMode.DoubleRow

            def head(g):
                """Score matmuls for group g + per-tile negated max.
                Each tile gets its OWN 1-bank PSUM tile and reduce so the
                buffer-release semaphore fires per tile: head(g+2) tile j
                only waits on exp(g) tile j, keeping the PSUM-recycle cycle
                well under 2x the engine-busy period."""
                gtok0 = g * TILE * GRP
                b = gtok0 // N
                for lb, s0, s1 in load_plan.get(g, ()):
                    load_slice(lb, s0, s1)
                psts = []
                nbmxs = []
                for j in range(GRP):
                    tok0 = gtok0 + j * TILE
                    o = tok0 % LOAD
                    xht, x8t = xslice(b, o, TILE)
                    ps = ps_s.tile([TILE, M], F32, tag="pst")
                    nc.tensor.matmul(out=ps, lhsT=xht, rhs=mh16,
                                     start=True, stop=False)
                    nc.tensor.matmul(out=ps, lhsT=x8t[:, 0:2, :],
                                     rhs=m8[:, 0:2, :],
                                     start=False, stop=False, perf_mode=DR)
                    nc.tensor.matmul(out=ps, lhsT=x8t[:, 2:4, :],
                                     rhs=m8[:, 2:4, :],
                                     start=False, stop=True, perf_mode=DR)
                    nbmx = spool.tile([TILE, 1], F32, tag="nbmx")
                    nc.vector.reduce_max(
                        out=nbmx, in_=ps, axis=mybir.AxisListType.X,
                        negate=True,
                    )
                    psts.append(ps)
                    nbmxs.append(nbmx)
                return psts, nbmxs

            def exp_stage(g, psts, nbmxs):
                """Exp one-hot for group g (1 group late): both tiles write
                fp8 bytes into one shared fp16-typed pack tile (tile j owns
                byte plane j)."""
                ohp = ohpool.tile([TILE, M], F16)
                oh8 = ohp.bitcast(F8).rearrange("p (m two) -> p m two", two=2)
                for j in range(GRP):
                    nc.scalar.activation(
                        out=oh8[:, :, j], in_=psts[j], func=AF.Exp,
                        bias=nbmxs[j], scale=1.0,
                    )
                return ohp

            def tr_stage(g, ohp):
                """4 fp16 transposes move both byte planes at once (2 groups
                late) into a per-group 1-bank PSUM tile."""
                psT = ps_t.tile([TILE, KCH, TILE], F16, tag="psT")
                for k in range(KCH):
                    nc.tensor.transpose(
                        out=psT[:, k, :],
                        in_=ohp[:, k * TILE : (k + 1) * TILE],
                        identity=idu,
                    )
                return psT

            def copy_stage(psT, g):
                """Per-group copy (DVE 2x fp16 mode; every 8th on ACT to
                shave the DVE bottleneck) moves the transposed one-hots to
                SBUF, one iteration after the transposes. (GPSIMD cannot
                access PSUM, so DVE/ACT carry all copies.)"""
                ohT = ohtpool.tile([TILE, KCH, TILE], F16)
                if g % 5 == 3:
                    nc.scalar.activation(out=ohT, in_=psT, func=AF.Copy)
                else:
                    nc.vector.tensor_copy(ohT, psT)
                return ohT

            po_cur = [None]

            def gather_stage(g, ohT, half):
                """fp8 DoubleRow gather (3 groups late): per tile 4 DR
                matmuls (chunk-pair x hi/lo). Pairs of groups share one
                [C, 512] fp32 PSUM bank."""
                if g % 2 == 0:
                    po_tile = ps_o.tile([C, STORE], F32, tag="po")
                    po_cur[0] = po_tile
                po = po_cur[0]
                ohT8 = ohT.bitcast(F8).rearrange(
                    "p k (t two) -> p k t two", two=2
                )
                for j in range(GRP):
                    col0 = (g % 2) * GRP * TILE + j * TILE
                    n_mm = 0
                    for pr in range(KCH // 2):
                        for h in range(2):
                            nc.tensor.matmul(
                                out=po[:, col0 : col0 + TILE],
                                lhsT=g8v[:, 2 * pr : 2 * pr + 2, h, :],
                                rhs=ohT8[:, 2 * pr : 2 * pr + 2, :, j],
                                start=(n_mm == 0), stop=(n_mm == 2 * KCH - 1),
                                perf_mode=DR,
                            )
                            n_mm += 1
                return po if g % 2 == 1 else None

            def tail(g, po):
                """Batched out-copy on ACT (5 groups late): [C, 512] fp32
                PSUM -> fp16 SBUF, then one DMA per STORE tokens."""
                gtok0 = g * TILE * GRP
                ob = opool.tile([C, STORE], F16, tag="ob")
                nc.scalar.activation(out=ob, in_=po, func=AF.Copy)
                b, n0 = divmod(gtok0 + GRP * TILE - STORE, N)
                nc.sync.dma_start(out=out_d[b, :, n0 : n0 + STORE], in_=ob)

            # Software pipeline, one iteration per score group g:
            #   PE : tr(g-2) | gather(g-5) | head(g)     (ready work first;
            #        head's buf wait is the in-order SEQ block point)
            #   DVE: ohT copy (pair (g-4)//2) | reduce(g)
            #   ACT: exps(g-1) | tail copy(g-6)
            # The head->reduce->exp->head PSUM recycle is the critical cycle;
            # per-TILE subtile deps (exp tile j releases the score buf slice
            # for head(g+2) tile j) keep it under 2x the engine-busy period.
            p_head = None
            exps = {}
            psTs = {}
            ohts = {}
            pos = {}
            for g in range(n_groups + 8):
                # Per-iteration stages: head(g) | exp(g-1) | tr(g-2) |
                # copy(g-3) | gather(g-4) | tail(g-6, per pair). Every
                # cross-engine dependency is >=1 iteration old; the score-buf
                # recycle (head(g+2) tile j <- exp(g) tile j) is the
                # critical cycle, kept short by per-tile PSUM tiles.
                g3 = g - 3
                if g3 >= 0 and g3 in psTs:
                    ohts[g3] = copy_stage(psTs.pop(g3), g3)
                nxt_head = head(g) if g < n_groups else None
                if g - 2 >= 0 and (g - 2) in exps:
                    psTs[g - 2] = tr_stage(g - 2, exps.pop(g - 2))
                g4 = g - 4
                if g4 >= 0 and g4 in ohts:
                    po = gather_stage(g4, ohts.pop(g4), half=g4 % 2)
                    if po is not None:
                        pos[g4 // 2] = po
                if p_head is not None:
                    exps[g - 1] = exp_stage(g - 1, *p_head)
                g6 = g - 6
                if g6 >= 0 and g6 % 2 == 1 and (g6 // 2) in pos:
                    tail(g6, pos.pop(g6 // 2))
                p_head = nxt_head

    _legalize_waits(nc)
    nc.finalize()
    return nc


def _legalize_waits(nc):
    """This container's walrus accepts only ONE sync wait per engine
    instruction (setupSyncWait: 'Too many sync wait commands'). Tile emits
    multi-wait instructions (and an 11-wait tail drain). Split: keep one
    wait on the instruction, hoist the rest onto single-wait Drain ops
    inserted just before it on the same engine (engine order preserved =>
    semantics preserved). DMA copies are left alone (ring descriptors
    accept multiple waits)."""
    n_split = 0
    for f in nc.m.functions:
        for b in f.blocks:
            out = []
            for inst in b.instructions:
                si = inst.sync_info
                if si is not None and len(si.on_wait) > 1:
                    waits = list(si.on_wait)
                    for j, w in enumerate(waits[:-1]):
                        out.append(
                            mybir.InstDrain(
                                name=f"{inst.name}-w{j}",
                                engine=inst.engine,
                                ins=[],
                                outs=[],
                                sync_info=mybir.SyncInfo(
                                    on_wait=[w], on_update=[]
                                ),
                            )
                        )
                    inst.sync_info = mybir.SyncInfo(
                        on_wait=[waits[-1]], on_update=list(si.on_update)
                    )
                    n_split += 1
                out.append(inst)
            b.instructions = out
    return n_split


_NC = None


def _get_nc():
    global _NC
    if _NC is None:
        _NC = _build()
    return _NC


def _host_prep(x, memory):
    import ml_dtypes
    f8 = ml_dtypes.float8_e4m3

    memn = memory / np.maximum(
        np.sqrt((memory * memory).sum(axis=1, keepdims=True)), 1e-12
    )
    # Scale the normalized memory by BETA so PSUM holds beta*s directly
    # (exp sharpness K_eff = beta ~ 1e5). Cap keeps fp16 mh finite.
    beta = min(1e5, 55000.0 / max(float(np.abs(memn).max()), 1e-6))
    mnt = np.ascontiguousarray(memn.T).astype(np.float32) * beta   # [C, M]
    mh = mnt.astype(np.float16)
    ml = (mnt - mh.astype(np.float32)).astype(np.float32)

    # DR rhs slot tables (all exact power-of-2 shifts of fp8 encodings)
    mhf = mh.astype(np.float32)
    m8 = np.zeros((C, 4, M), dtype=f8)
    m8[:, 0, :] = (mhf / SIG).astype(f8)             # mhA
    m8[:, 1, :] = (mhf / (SIG * 16.0)).astype(f8)    # mhB
    m8[:, 2, :] = ml.astype(f8)                      # mlA
    m8[:, 3, :] = (ml / 16.0).astype(f8)             # mlB

    xh = x.astype(np.float16)
    xl = (x - xh.astype(np.float32)).astype(np.float32)
    xl1 = (xl * SIG).astype(f8)
    xl2 = ((xl * SIG - xl1.astype(np.float32)) * 16.0).astype(f8)
    xh8a = xh.astype(np.float32).astype(f8)
    xh8b = ((xh.astype(np.float32) - xh8a.astype(np.float32)) * 16.0).astype(f8)
    x8 = np.stack([xl1, xl2, xh8a, xh8b], axis=2)    # [B, C, 4, HW...]

    gh8 = memory.astype(f8)
    gl8 = (memory - gh8.astype(np.float32)).astype(f8)
    g8 = np.zeros((TILE, KCH * 2 * C), dtype=f8)
    for k in range(KCH):
        base = k * 2 * C
        g8[:, base : base + C] = gh8[k * TILE : (k + 1) * TILE, :]
        g8[:, base + C : base + 2 * C] = gl8[k * TILE : (k + 1) * TILE, :]

    return xh, x8, mh, m8, g8


def kernel(x, memory):
    x = np.asarray(x, dtype=np.float32)
    memory = np.asarray(memory, dtype=np.float32)
    nc = _get_nc()
    xf = x.reshape(B, C, N)
    xh, x8, mh16, m8, g8 = _host_prep(xf, memory)
    idu = np.eye(TILE, dtype=np.float16)

    in_maps = []
    for c in range(NCORES):
        in_maps.append({
            "xh": np.ascontiguousarray(xh[c * BPC : (c + 1) * BPC]),
            "x8": np.ascontiguousarray(x8[c * BPC : (c + 1) * BPC]),
            "mh16": mh16, "m8": m8, "g8": g8, "idu": idu,
        })

    res = run_bass_kernel_spmd(nc, in_maps, core_ids=list(range(NCORES)))
    outs = [
        r["out"].astype(np.float32).reshape(BPC, C, H, W) for r in res.results
    ]
    return np.concatenate(outs, axis=0)


# revision 19
# speedup vs baseline: 1.2817x; 1.0314x over previous
"""Trainium2 Bass kernel for nn_HardMemory (retrieval_knn).

For each spatial token (B*H*W tokens, C=128 channels), find the memory row
(of M=512) with max cosine similarity and replace the token's channel vector
with that raw memory row.

Algebraic simplification: argmax_m cos(x, mem_m) = argmax_m (x . mem_n_m)
where mem_n is the l2-normalized memory -- normalizing x is a positive
per-token scale and cannot change the argmax, so it is skipped.

Scores (PSUM fp32, beta-scaled): s = xh.mh16 (one fp16 matmul) plus two fp8
DoubleRow matmuls carrying the precision cross-terms at half cost:
  DR-A: (xl1, xl2).(mhA, mhB)   ~= xl.mh      (xl = fp16 residual of x,
        split in two e4m3 terms pre-scaled by 2^16; mh slots down-scaled)
  DR-B: (xh8a, xh8b).(mlA, mlB) ~= xh.ml      (ml = fp16 residual of the
        beta-scaled memory; xh split in two e4m3 terms)
Each DoubleRow sums TWO independent 128-deep products at 0.5 cycles/row, so
per-tile score cost is 512+256+256 = 1024 PE cycles vs 1536 for the 3-term
fp16 scheme. Dropped terms are O(2^-15) of score scale; measured ~10 argmax
flips over 131072 tokens, well under the 2e-2 rel-err gate.

One-hot: exp(s - max) on ACT (bias = negated reduce_max, scale=1), written
as e4m3 bytes at stride 2 into a SHARED fp16 tile: tile j of the group owns
byte plane j. One fp16 128x128 transpose then moves BOTH tiles' one-hot
bytes at once (one-hot bytes are <= 0x38 so the fp16 lanes are always
finite positive and the identity-matmul transpose is exact). This halves
both the PE transpose cost and the PSUM->SBUF copy volume vs per-tile fp8
transposes.

The gather reconstructs memory rows from a 2-term fp8 (e4m3 hi+lo) split
via fp8 DoubleRow one-hot matmuls reading stride-2 byte views of the
transposed pack: per tile 4 DR matmuls (2 chunk-pairs x hi/lo) at 0.5
cycles/row. Output fp32 PSUM -> fp16 SBUF copy -> DMA, stored fp16 and
upcast on the host (fp8 recon err ~1e-3 rel, far under the gate).

Hard constraints that shape the engine assignment (walrus-verified):
  - GPSIMD/Pool cannot access PSUM at all -> every PSUM-side op (reduce,
    exp, one-hot copies, out-copies) must share DVE + ACT only.
  - Free-axis reduce_max is DVE-only and has NO 2x perf mode (1 elem/cyc
    regardless of dtype); an engine may read only ONE non-scalar operand
    from PSUM (so no tensor_tensor max-of-halves trick on PSUM scores).
  - DMA cannot read PSUM, and matmul output must be fp32 PSUM, so the
    gather out-copy is unavoidable engine work.
Engine busy per group (2 tiles, 256 tokens), 64 groups per core:
  PE  : scores 853ns + transposes 213ns + gather 213ns   = 1280ns
  DVE : 2 per-tile reduce_max 658ns + ~4/5 ohT copies    = ~1640ns
  ACT : 2 exps 612ns + tail copy 306ns + ~1/5 ohT copies = ~1660ns
DVE/ACT are the bottleneck pair (~105us busy of ~128us span); the
remainder is cross-engine semaphore latency and pipeline fill/drain.

Pipelining: per-TILE PSUM score tiles (1 bank each, 4 bufs) make the
buffer-release semaphore fire per tile, so head(g+2) tile j waits only on
exp(g) tile j (612ns earlier than the group's second exp). With a shared
2-tile score tile the release counts BOTH exp reads and the recycle chain
head->reduce->exp->head serializes to a ~2.4us period (measured); per-tile
tiles bring the period down to the DVE/ACT busy floor.

Sharding: data-parallel over batch, 4 batches per core, memory replicated.
Input DMAs are sliced and spread across groups so the 625ns HWDGE
descriptor setups never serialize against compute.
"""

import numpy as np

import concourse.bass as bass
import concourse.mybir as mybir
from concourse.tile import TileContext
from concourse.bass_utils import run_bass_kernel_spmd

F32 = mybir.dt.float32
F16 = mybir.dt.float16
F8 = mybir.dt.float8e4
AF = mybir.ActivationFunctionType

B, C, H, W = 32, 128, 64, 64
N = H * W              # 4096 tokens per batch
M = 512                # memory rows
NCORES = 8
BPC = B // NCORES      # batches per core
TOK = BPC * N          # tokens per core
TILE = 128             # tokens per tile
GRP = 2                # tiles per PSUM score group
LOAD = 4096            # tokens per input DMA region (one full batch image)
STORE = 512            # tokens per output DMA chunk (2 groups)
KCH = M // TILE        # 4 gather chunks
SIG = 65536.0          # xl pre-scale for the fp8 split (exact power of 2)


def _build():
    nc = bass.Bass(trn_type="TRN2")

    xh_in = nc.dram_tensor("xh", [BPC, C, N], F16, kind="ExternalInput")
    # fp8 slots: 0=xl1 1=xl2 (DR-A lhsT pair), 2=xh8a 3=xh8b (DR-B pair)
    x8_in = nc.dram_tensor("x8", [BPC, C, 4, N], F8, kind="ExternalInput")
    mh_in = nc.dram_tensor("mh16", [C, M], F16, kind="ExternalInput")
    # fp8 slots: 0=mhA 1=mhB (DR-A rhs pair), 2=mlA 3=mlB (DR-B rhs pair)
    m8_in = nc.dram_tensor("m8", [C, 4, M], F8, kind="ExternalInput")
    # raw memory rows fp8 hi/lo [TILE, KCH, 2, C] packed
    g8_in = nc.dram_tensor("g8", [TILE, KCH * 2 * C], F8, kind="ExternalInput")
    idu_in = nc.dram_tensor("idu", [TILE, TILE], F16, kind="ExternalInput")
    out_d = nc.dram_tensor("out", [BPC, C, N], F16, kind="ExternalOutput")

    with TileContext(nc) as tc:
        with (
            tc.tile_pool(name="const", bufs=1) as cpool,
            tc.tile_pool(name="xin", bufs=4) as xpool,
            tc.tile_pool(name="oh", bufs=6) as ohpool,
            tc.tile_pool(name="oht", bufs=2) as ohtpool,
            tc.tile_pool(name="osb", bufs=4) as opool,
            tc.tile_pool(name="small", bufs=12) as spool,
            tc.tile_pool(name="ps_s", bufs=4, space="PSUM") as ps_s,
            tc.tile_pool(name="ps_t", bufs=2, space="PSUM") as ps_t,
            tc.tile_pool(name="ps_o", bufs=2, space="PSUM") as ps_o,
        ):
            n_groups = TOK // (TILE * GRP)
            grp_per_batch = N // (TILE * GRP)
            loaded = {}

            def load_slice(b, s0, s1):
                nb = 8 if s1 - s0 == 1024 else 1
                xh_sb = xpool.tile(
                    [C, s1 - s0], F16, tag=f"xh{s1 - s0}", bufs=nb
                )
                nc.sync.dma_start(out=xh_sb, in_=xh_in[b, :, s0:s1])
                x8_sb = xpool.tile(
                    [C, 4, s1 - s0], F8, tag=f"x8{s1 - s0}", bufs=nb
                )
                nc.sync.dma_start(out=x8_sb, in_=x8_in[b, :, :, s0:s1])
                loaded.setdefault(b, []).append((s0, s1, xh_sb, x8_sb))

            def xslice(b, o, size):
                for s0, s1, xh_sb, x8_sb in loaded[b]:
                    if s0 <= o and o + size <= s1:
                        return (xh_sb[:, o - s0 : o - s0 + size],
                                x8_sb[:, :, o - s0 : o - s0 + size])
                raise AssertionError((b, o, size))

            # Prefetch schedule: batches 1..3 load one 1024-token slice per
            # group, starting 12 groups before the batch is needed.
            load_plan = {}
            for b in range(1, BPC):
                for si in range(4):
                    load_plan.setdefault(
                        b * grp_per_batch - 12 + 2 * si, []
                    ).append((b, si * 1024, (si + 1) * 1024))

            mh16 = cpool.tile([C, M], F16)
            nc.sync.dma_start(out=mh16, in_=mh_in[:])
            m8 = cpool.tile([C, 4, M], F8)
            nc.sync.dma_start(out=m8, in_=m8_in[:])
            # batch 0: small leading slices so PE starts early
            for s0, s1 in ((0, 256), (256, 1024), (1024, 2048), (2048, 3072),
                           (3072, N)):
                load_slice(0, s0, s1)
            g8 = cpool.tile([TILE, KCH * 2 * C], F8)
            nc.sync.dma_start(out=g8, in_=g8_in[:])
            g8v = g8.rearrange("p (k h c) -> p k h c", k=KCH, h=2)
            idu = cpool.tile([TILE, TILE], F16)
            nc.sync.dma_start(out=idu, in_=idu_in[:])

            DR = mybir.MatmulPerfMode.DoubleRow

            def head(g):
                """Score matmuls for group g + per-tile negated max.
                Each tile gets its OWN 1-bank PSUM tile and reduce so the
                buffer-release semaphore fires per tile: head(g+2) tile j
                only waits on exp(g) tile j, keeping the PSUM-recycle cycle
                well under 2x the engine-busy period."""
                gtok0 = g * TILE * GRP
                b = gtok0 // N
                for lb, s0, s1 in load_plan.get(g, ()):
                    load_slice(lb, s0, s1)
                psts = []
                nbmxs = []
                for j in range(GRP):
                    tok0 = gtok0 + j * TILE
                    o = tok0 % LOAD
                    xht, x8t = xslice(b, o, TILE)
                    ps = ps_s.tile([TILE, M], F32, tag="pst")
                    nc.tensor.matmul(out=ps, lhsT=xht, rhs=mh16,
                                     start=True, stop=False)
                    nc.tensor.matmul(out=ps, lhsT=x8t[:, 0:2, :],
                                     rhs=m8[:, 0:2, :],
                                     start=False, stop=False, perf_mode=DR)
                    nc.tensor.matmul(out=ps, lhsT=x8t[:, 2:4, :],
                                     rhs=m8[:, 2:4, :],
                                     start=False, stop=True, perf_mode=DR)
                    nbmx = spool.tile([TILE, 1], F32, tag="nbmx")
                    nc.vector.reduce_max(
                        out=nbmx, in_=ps, axis=mybir.AxisListType.X,
                        negate=True,
                    )
                    psts.append(ps)
                    nbmxs.append(nbmx)
                return psts, nbmxs

            def exp_stage(g, psts, nbmxs):
                """Exp one-hot for group g (1 group late): both tiles write
                fp8 bytes into one shared fp16-typed pack tile (tile j owns
                byte plane j)."""
                ohp = ohpool.tile([TILE, M], F16)
                oh8 = ohp.bitcast(F8).rearrange("p (m two) -> p m two", two=2)
                for j in range(GRP):
                    nc.scalar.activation(
                        out=oh8[:, :, j], in_=psts[j], func=AF.Exp,
                        bias=nbmxs[j], scale=1.0,
                    )
                return ohp

            def tr_stage(g, ohp):
                """4 fp16 transposes move both byte planes at once (2 groups
                late) into a per-group 1-bank PSUM tile."""
                psT = ps_t.tile([TILE, KCH, TILE], F16, tag="psT")
                for k in range(KCH):
                    nc.tensor.transpose(
                        out=psT[:, k, :],
                        in_=ohp[:, k * TILE : (k + 1) * TILE],
                        identity=idu,
                    )
                return psT

            def copy_stage(psT, g):
                """Per-group copy (DVE 2x fp16 mode; every 8th on ACT to
                shave the DVE bottleneck) moves the transposed one-hots to
                SBUF, one iteration after the transposes. (GPSIMD cannot
                access PSUM, so DVE/ACT carry all copies.)"""
                ohT = ohtpool.tile([TILE, KCH, TILE], F16)
                if g % 4 == 3:
                    nc.scalar.activation(out=ohT, in_=psT, func=AF.Copy)
                else:
                    nc.vector.tensor_copy(ohT, psT)
                return ohT

            po_cur = [None]

            def gather_stage(g, ohT, half):
                """fp8 DoubleRow gather (3 groups late): per tile 4 DR
                matmuls (chunk-pair x hi/lo). Pairs of groups share one
                [C, 512] fp32 PSUM bank."""
                if g % 2 == 0:
                    po_tile = ps_o.tile([C, STORE], F32, tag="po")
                    po_cur[0] = po_tile
                po = po_cur[0]
                ohT8 = ohT.bitcast(F8).rearrange(
                    "p k (t two) -> p k t two", two=2
                )
                for j in range(GRP):
                    col0 = (g % 2) * GRP * TILE + j * TILE
                    n_mm = 0
                    for pr in range(KCH // 2):
                        for h in range(2):
                            nc.tensor.matmul(
                                out=po[:, col0 : col0 + TILE],
                                lhsT=g8v[:, 2 * pr : 2 * pr + 2, h, :],
                                rhs=ohT8[:, 2 * pr : 2 * pr + 2, :, j],
                                start=(n_mm == 0), stop=(n_mm == 2 * KCH - 1),
                                perf_mode=DR,
                            )
                            n_mm += 1
                return po if g % 2 == 1 else None

            def tail(g, po):
                """Batched out-copy on ACT (5 groups late): [C, 512] fp32
                PSUM -> fp16 SBUF, then one DMA per STORE tokens."""
                gtok0 = g * TILE * GRP
                ob = opool.tile([C, STORE], F16, tag="ob")
                nc.scalar.activation(out=ob, in_=po, func=AF.Copy)
                b, n0 = divmod(gtok0 + GRP * TILE - STORE, N)
                nc.sync.dma_start(out=out_d[b, :, n0 : n0 + STORE], in_=ob)

            # Software pipeline, one iteration per score group g:
            #   PE : tr(g-2) | gather(g-5) | head(g)     (ready work first;
            #        head's buf wait is the in-order SEQ block point)
            #   DVE: ohT copy (pair (g-4)//2) | reduce(g)
            #   ACT: exps(g-1) | tail copy(g-6)
            # The head->reduce->exp->head PSUM recycle is the critical cycle;
            # per-TILE subtile deps (exp tile j releases the score buf slice
            # for head(g+2) tile j) keep it under 2x the engine-busy period.
            p_head = None
            exps = {}
            psTs = {}
            ohts = {}
            pos = {}
            for g in range(n_groups + 8):
                # Per-iteration stages: head(g) | exp(g-1) | tr(g-2) |
                # copy(g-3) | gather(g-4) | tail(g-6, per pair). Every
                # cross-engine dependency is >=1 iteration old; the score-buf
                # recycle (head(g+2) tile j <- exp(g) tile j) is the
                # critical cycle, kept short by per-tile PSUM tiles.
                g3 = g - 3
                if g3 >= 0 and g3 in psTs:
                    ohts[g3] = copy_stage(psTs.pop(g3), g3)
                nxt_head = head(g) if g < n_groups else None
                if g - 2 >= 0 and (g - 2) in exps:
                    psTs[g - 2] = tr_stage(g - 2, exps.pop(g - 2))
                g4 = g - 4
                if g4 >= 0 and g4 in ohts:
                    po = gather_stage(g4, ohts.pop(g4), half=g4 % 2)
                    if po is not None:
                        pos[g4 // 2] = po
                if p_head is not None:
                    exps[g - 1] = exp_stage(g - 1, *p_head)
                g6 = g - 6
                if g6 >= 0 and g6 % 2 == 1 and (g6 // 2) in pos:
                    tail(g6, pos.pop(g6 // 2))
                p_head = nxt_head

    _legalize_waits(nc)
    nc.finalize()
    return nc


def _legalize_waits(nc):
    """This container's walrus accepts only ONE sync wait per engine
    instruction (setupSyncWait: 'Too many sync wait commands'). Tile emits
    multi-wait instructions (and an 11-wait tail drain). Split: keep one
    wait on the instruction, hoist the rest onto single-wait Drain ops
    inserted just before it on the same engine (engine order preserved =>
    semantics preserved). DMA copies are left alone (ring descriptors
    accept multiple waits)."""
    n_split = 0
    for f in nc.m.functions:
        for b in f.blocks:
            out = []
            for inst in b.instructions:
                si = inst.sync_info
                if si is not None and len(si.on_wait) > 1:
                    waits = list(si.on_wait)
                    for j, w in enumerate(waits[:-1]):
                        out.append(
                            mybir.InstDrain(
                                name=f"{inst.name}-w{j}",
                                engine=inst.engine,
                                ins=[],
                                outs=[],
                                sync_info=mybir.SyncInfo(
                                    on_wait=[w], on_update=[]
                                ),
                            )
                        )
                    inst.sync_info = mybir.SyncInfo(
                        on_wait=[waits[-1]], on_update=list(si.on_update)
                    )
                    n_split += 1
                out.append(inst)
            b.instructions = out
    return n_split


_NC = None


def _get_nc():
    global _NC
    if _NC is None:
        _NC = _build()
    return _NC


def _host_prep(x, memory):
    import ml_dtypes
    f8 = ml_dtypes.float8_e4m3

    memn = memory / np.maximum(
        np.sqrt((memory * memory).sum(axis=1, keepdims=True)), 1e-12
    )
    # Scale the normalized memory by BETA so PSUM holds beta*s directly
    # (exp sharpness K_eff = beta ~ 1e5). Cap keeps fp16 mh finite.
    beta = min(1e5, 55000.0 / max(float(np.abs(memn).max()), 1e-6))
    mnt = np.ascontiguousarray(memn.T).astype(np.float32) * beta   # [C, M]
    mh = mnt.astype(np.float16)
    ml = (mnt - mh.astype(np.float32)).astype(np.float32)

    # DR rhs slot tables (all exact power-of-2 shifts of fp8 encodings)
    mhf = mh.astype(np.float32)
    m8 = np.zeros((C, 4, M), dtype=f8)
    m8[:, 0, :] = (mhf / SIG).astype(f8)             # mhA
    m8[:, 1, :] = (mhf / (SIG * 16.0)).astype(f8)    # mhB
    m8[:, 2, :] = ml.astype(f8)                      # mlA
    m8[:, 3, :] = (ml / 16.0).astype(f8)             # mlB

    xh = x.astype(np.float16)
    xl = (x - xh.astype(np.float32)).astype(np.float32)
    xl1 = (xl * SIG).astype(f8)
    xl2 = ((xl * SIG - xl1.astype(np.float32)) * 16.0).astype(f8)
    xh8a = xh.astype(np.float32).astype(f8)
    xh8b = ((xh.astype(np.float32) - xh8a.astype(np.float32)) * 16.0).astype(f8)
    x8 = np.stack([xl1, xl2, xh8a, xh8b], axis=2)    # [B, C, 4, HW...]

    gh8 = memory.astype(f8)
    gl8 = (memory - gh8.astype(np.float32)).astype(f8)
    g8 = np.zeros((TILE, KCH * 2 * C), dtype=f8)
    for k in range(KCH):
        base = k * 2 * C
        g8[:, base : base + C] = gh8[k * TILE : (k + 1) * TILE, :]
        g8[:, base + C : base + 2 * C] = gl8[k * TILE : (k + 1) * TILE, :]

    return xh, x8, mh, m8, g8


def kernel(x, memory):
    x = np.asarray(x, dtype=np.float32)
    memory = np.asarray(memory, dtype=np.float32)
    nc = _get_nc()
    xf = x.reshape(B, C, N)
    xh, x8, mh16, m8, g8 = _host_prep(xf, memory)
    idu = np.eye(TILE, dtype=np.float16)

    in_maps = []
    for c in range(NCORES):
        in_maps.append({
            "xh": np.ascontiguousarray(xh[c * BPC : (c + 1) * BPC]),
            "x8": np.ascontiguousarray(x8[c * BPC : (c + 1) * BPC]),
            "mh16": mh16, "m8": m8, "g8": g8, "idu": idu,
        })

    res = run_bass_kernel_spmd(nc, in_maps, core_ids=list(range(NCORES)))
    outs = [
        r["out"].astype(np.float32).reshape(BPC, C, H, W) for r in res.results
    ]
    return np.concatenate(outs, axis=0)


# revision 22
# speedup vs baseline: 1.3021x; 1.0159x over previous
"""Trainium2 Bass kernel for nn_HardMemory (retrieval_knn).

For each spatial token (B*H*W tokens, C=128 channels), find the memory row
(of M=512) with max cosine similarity and replace the token's channel vector
with that raw memory row.

Algebraic simplification: argmax_m cos(x, mem_m) = argmax_m (x . mem_n_m)
where mem_n is the l2-normalized memory -- normalizing x is a positive
per-token scale and cannot change the argmax, so it is skipped.

Scores (PSUM fp32, beta-scaled): s = xh.mh16 (one fp16 matmul) plus two fp8
DoubleRow matmuls carrying the precision cross-terms at half cost:
  DR-A: (xl1, xl2).(mhA, mhB)   ~= xl.mh      (xl = fp16 residual of x,
        split in two e4m3 terms pre-scaled by 2^16; mh slots down-scaled)
  DR-B: (xh8a, xh8b).(mlA, mlB) ~= xh.ml      (ml = fp16 residual of the
        beta-scaled memory; xh split in two e4m3 terms)
Each DoubleRow sums TWO independent 128-deep products at 0.5 cycles/row, so
per-tile score cost is 512+256+256 = 1024 PE cycles vs 1536 for the 3-term
fp16 scheme. Dropped terms are O(2^-15) of score scale; measured ~10 argmax
flips over 131072 tokens, well under the 2e-2 rel-err gate.

One-hot: exp(s - max) on ACT (bias = negated reduce_max, scale=1), written
as e4m3 bytes at stride 2 into a SHARED fp16 tile: tile j of the group owns
byte plane j. One fp16 128x128 transpose then moves BOTH tiles' one-hot
bytes at once (one-hot bytes are <= 0x38 so the fp16 lanes are always
finite positive and the identity-matmul transpose is exact). This halves
both the PE transpose cost and the PSUM->SBUF copy volume vs per-tile fp8
transposes.

The gather reconstructs memory rows from a 2-term fp8 (e4m3 hi+lo) split
via fp8 DoubleRow one-hot matmuls reading stride-2 byte views of the
transposed pack: per tile 4 DR matmuls (2 chunk-pairs x hi/lo) at 0.5
cycles/row. Output fp32 PSUM -> fp16 SBUF copy -> DMA, stored fp16 and
upcast on the host (fp8 recon err ~1e-3 rel, far under the gate).

Hard constraints that shape the engine assignment (walrus-verified):
  - GPSIMD/Pool cannot access PSUM at all -> every PSUM-side op (reduce,
    exp, one-hot copies, out-copies) must share DVE + ACT only.
  - Free-axis reduce_max is DVE-only and has NO 2x perf mode (1 elem/cyc
    regardless of dtype); an engine may read only ONE non-scalar operand
    from PSUM (so no tensor_tensor max-of-halves trick on PSUM scores).
  - DMA cannot read PSUM, and matmul output must be fp32 PSUM, so the
    gather out-copy is unavoidable engine work.
Engine busy per group (2 tiles, 256 tokens), 64 groups per core:
  PE  : scores 853ns + transposes 213ns + gather 213ns   = 1280ns
  DVE : 2 per-tile reduce_max 658ns + ~3/4 ohT copies    = ~1610ns
  ACT : 2 exps 612ns + tail copy 306ns + ~1/4 ohT copies = ~1680ns
DVE/ACT are the bottleneck pair (~90% busy in steady state of the
~124us span); the remainder is cross-engine semaphore latency and
pipeline fill/drain.

Pipelining: per-TILE PSUM score tiles (1 bank each, 4 bufs) make the
buffer-release semaphore fire per tile, so head(g+2) tile j waits only on
exp(g) tile j (612ns earlier than the group's second exp). With a shared
2-tile score tile the release counts BOTH exp reads and the recycle chain
head->reduce->exp->head serializes to a ~2.4us period (measured); per-tile
tiles bring the period down to the DVE/ACT busy floor.

Sharding: data-parallel over batch, 4 batches per core, memory replicated.
Input DMAs are sliced and spread across groups so the 625ns HWDGE
descriptor setups never serialize against compute.
"""

import numpy as np

import concourse.bass as bass
import concourse.mybir as mybir
from concourse.tile import TileContext
from concourse.bass_utils import run_bass_kernel_spmd

F32 = mybir.dt.float32
F16 = mybir.dt.float16
F8 = mybir.dt.float8e4
AF = mybir.ActivationFunctionType

B, C, H, W = 32, 128, 64, 64
N = H * W              # 4096 tokens per batch
M = 512                # memory rows
NCORES = 8
BPC = B // NCORES      # batches per core
TOK = BPC * N          # tokens per core
TILE = 128             # tokens per tile
GRP = 2                # tiles per PSUM score group
LOAD = 4096            # tokens per input DMA region (one full batch image)
STORE = 512            # tokens per output DMA chunk (2 groups)
KCH = M // TILE        # 4 gather chunks
SIG = 65536.0          # xl pre-scale for the fp8 split (exact power of 2)


def _build():
    nc = bass.Bass(trn_type="TRN2")

    xh_in = nc.dram_tensor("xh", [BPC, C, N], F16, kind="ExternalInput")
    # fp8 slots: 0=xl1 1=xl2 (DR-A lhsT pair), 2=xh8a 3=xh8b (DR-B pair)
    x8_in = nc.dram_tensor("x8", [BPC, C, 4, N], F8, kind="ExternalInput")
    mh_in = nc.dram_tensor("mh16", [C, M], F16, kind="ExternalInput")
    # fp8 slots: 0=mhA 1=mhB (DR-A rhs pair), 2=mlA 3=mlB (DR-B rhs pair)
    m8_in = nc.dram_tensor("m8", [C, 4, M], F8, kind="ExternalInput")
    # raw memory rows fp8 hi/lo [TILE, KCH, 2, C] packed
    g8_in = nc.dram_tensor("g8", [TILE, KCH * 2 * C], F8, kind="ExternalInput")
    idu_in = nc.dram_tensor("idu", [TILE, TILE], F16, kind="ExternalInput")
    out_d = nc.dram_tensor("out", [BPC, C, N], F16, kind="ExternalOutput")

    with TileContext(nc) as tc:
        with (
            tc.tile_pool(name="const", bufs=1) as cpool,
            tc.tile_pool(name="xin", bufs=4) as xpool,
            tc.tile_pool(name="oh", bufs=8) as ohpool,
            tc.tile_pool(name="oht", bufs=2) as ohtpool,
            tc.tile_pool(name="osb", bufs=4) as opool,
            tc.tile_pool(name="small", bufs=12) as spool,
            tc.tile_pool(name="ps_s", bufs=4, space="PSUM") as ps_s,
            tc.tile_pool(name="ps_t", bufs=2, space="PSUM") as ps_t,
            tc.tile_pool(name="ps_o", bufs=2, space="PSUM") as ps_o,
        ):
            n_groups = TOK // (TILE * GRP)
            grp_per_batch = N // (TILE * GRP)
            loaded = {}

            def load_slice(b, s0, s1):
                nb = 8 if s1 - s0 == 1024 else 1
                xh_sb = xpool.tile(
                    [C, s1 - s0], F16, tag=f"xh{s1 - s0}", bufs=nb
                )
                nc.sync.dma_start(out=xh_sb, in_=xh_in[b, :, s0:s1])
                x8_sb = xpool.tile(
                    [C, 4, s1 - s0], F8, tag=f"x8{s1 - s0}", bufs=nb
                )
                nc.sync.dma_start(out=x8_sb, in_=x8_in[b, :, :, s0:s1])
                loaded.setdefault(b, []).append((s0, s1, xh_sb, x8_sb))

            def xslice(b, o, size):
                for s0, s1, xh_sb, x8_sb in loaded[b]:
                    if s0 <= o and o + size <= s1:
                        return (xh_sb[:, o - s0 : o - s0 + size],
                                x8_sb[:, :, o - s0 : o - s0 + size])
                raise AssertionError((b, o, size))

            # Prefetch schedule: batches 1..3 load one 1024-token slice per
            # group, starting 12 groups before the batch is needed.
            load_plan = {}
            for b in range(1, BPC):
                for si in range(4):
                    load_plan.setdefault(
                        b * grp_per_batch - 12 + 2 * si, []
                    ).append((b, si * 1024, (si + 1) * 1024))

            mh16 = cpool.tile([C, M], F16)
            nc.sync.dma_start(out=mh16, in_=mh_in[:])
            m8 = cpool.tile([C, 4, M], F8)
            nc.sync.dma_start(out=m8, in_=m8_in[:])
            # batch 0: small leading slices so PE starts early
            for s0, s1 in ((0, 256), (256, 1024), (1024, 2048), (2048, 3072),
                           (3072, N)):
                load_slice(0, s0, s1)
            g8 = cpool.tile([TILE, KCH * 2 * C], F8)
            nc.sync.dma_start(out=g8, in_=g8_in[:])
            g8v = g8.rearrange("p (k h c) -> p k h c", k=KCH, h=2)
            idu = cpool.tile([TILE, TILE], F16)
            nc.sync.dma_start(out=idu, in_=idu_in[:])

            DR = mybir.MatmulPerfMode.DoubleRow

            def head(g):
                """Score matmuls for group g + per-tile negated max.
                Each tile gets its OWN 1-bank PSUM tile and reduce so the
                buffer-release semaphore fires per tile: head(g+2) tile j
                only waits on exp(g) tile j, keeping the PSUM-recycle cycle
                well under 2x the engine-busy period."""
                gtok0 = g * TILE * GRP
                b = gtok0 // N
                for lb, s0, s1 in load_plan.get(g, ()):
                    load_slice(lb, s0, s1)
                psts = []
                nbmxs = []
                for j in range(GRP):
                    tok0 = gtok0 + j * TILE
                    o = tok0 % LOAD
                    xht, x8t = xslice(b, o, TILE)
                    ps = ps_s.tile([TILE, M], F32, tag="pst")
                    nc.tensor.matmul(out=ps, lhsT=xht, rhs=mh16,
                                     start=True, stop=False)
                    nc.tensor.matmul(out=ps, lhsT=x8t[:, 0:2, :],
                                     rhs=m8[:, 0:2, :],
                                     start=False, stop=False, perf_mode=DR)
                    nc.tensor.matmul(out=ps, lhsT=x8t[:, 2:4, :],
                                     rhs=m8[:, 2:4, :],
                                     start=False, stop=True, perf_mode=DR)
                    nbmx = spool.tile([TILE, 1], F32, tag="nbmx")
                    nc.vector.reduce_max(
                        out=nbmx, in_=ps, axis=mybir.AxisListType.X,
                        negate=True,
                    )
                    psts.append(ps)
                    nbmxs.append(nbmx)
                return psts, nbmxs

            def exp_stage(g, psts, nbmxs):
                """Exp one-hot for group g (1 group late): both tiles write
                fp8 bytes into one shared fp16-typed pack tile (tile j owns
                byte plane j)."""
                ohp = ohpool.tile([TILE, M], F16)
                oh8 = ohp.bitcast(F8).rearrange("p (m two) -> p m two", two=2)
                for j in range(GRP):
                    nc.scalar.activation(
                        out=oh8[:, :, j], in_=psts[j], func=AF.Exp,
                        bias=nbmxs[j], scale=1.0,
                    )
                return ohp

            def tr_stage(g, ohp):
                """4 fp16 transposes move both byte planes at once (2 groups
                late) into a per-group 1-bank PSUM tile."""
                psT = ps_t.tile([TILE, KCH, TILE], F16, tag="psT")
                for k in range(KCH):
                    nc.tensor.transpose(
                        out=psT[:, k, :],
                        in_=ohp[:, k * TILE : (k + 1) * TILE],
                        identity=idu,
                    )
                return psT

            def copy_stage(psT, g):
                """Per-group copy (DVE 2x fp16 mode; every 8th on ACT to
                shave the DVE bottleneck) moves the transposed one-hots to
                SBUF, one iteration after the transposes. (GPSIMD cannot
                access PSUM, so DVE/ACT carry all copies.)"""
                ohT = ohtpool.tile([TILE, KCH, TILE], F16)
                if g % 4 == 0:
                    nc.scalar.activation(out=ohT, in_=psT, func=AF.Copy)
                else:
                    nc.vector.tensor_copy(ohT, psT)
                return ohT

            po_cur = [None]

            def gather_stage(g, ohT, half):
                """fp8 DoubleRow gather (3 groups late): per tile 4 DR
                matmuls (chunk-pair x hi/lo). Pairs of groups share one
                [C, 512] fp32 PSUM bank."""
                if g % 2 == 0:
                    po_tile = ps_o.tile([C, STORE], F32, tag="po")
                    po_cur[0] = po_tile
                po = po_cur[0]
                ohT8 = ohT.bitcast(F8).rearrange(
                    "p k (t two) -> p k t two", two=2
                )
                for j in range(GRP):
                    col0 = (g % 2) * GRP * TILE + j * TILE
                    n_mm = 0
                    for pr in range(KCH // 2):
                        for h in range(2):
                            nc.tensor.matmul(
                                out=po[:, col0 : col0 + TILE],
                                lhsT=g8v[:, 2 * pr : 2 * pr + 2, h, :],
                                rhs=ohT8[:, 2 * pr : 2 * pr + 2, :, j],
                                start=(n_mm == 0), stop=(n_mm == 2 * KCH - 1),
                                perf_mode=DR,
                            )
                            n_mm += 1
                return po if g % 2 == 1 else None

            def tail(g, po):
                """Batched out-copy on ACT (5 groups late): [C, 512] fp32
                PSUM -> fp16 SBUF, then one DMA per STORE tokens."""
                gtok0 = g * TILE * GRP
                ob = opool.tile([C, STORE], F16, tag="ob")
                nc.scalar.activation(out=ob, in_=po, func=AF.Copy)
                b, n0 = divmod(gtok0 + GRP * TILE - STORE, N)
                nc.sync.dma_start(out=out_d[b, :, n0 : n0 + STORE], in_=ob)

            # Software pipeline, one iteration per score group g:
            #   PE : tr(g-2) | gather(g-5) | head(g)     (ready work first;
            #        head's buf wait is the in-order SEQ block point)
            #   DVE: ohT copy (pair (g-4)//2) | reduce(g)
            #   ACT: exps(g-1) | tail copy(g-6)
            # The head->reduce->exp->head PSUM recycle is the critical cycle;
            # per-TILE subtile deps (exp tile j releases the score buf slice
            # for head(g+2) tile j) keep it under 2x the engine-busy period.
            p_head = None
            exps = {}
            psTs = {}
            ohts = {}
            pos = {}
            for g in range(n_groups + 8):
                # Per-iteration stages: head(g) | exp(g-1) | tr(g-2) |
                # copy(g-3) | gather(g-4) | tail(g-6, per pair). Every
                # cross-engine dependency is >=1 iteration old; the score-buf
                # recycle (head(g+2) tile j <- exp(g) tile j) is the
                # critical cycle, kept short by per-tile PSUM tiles.
                g3 = g - 3
                if g3 >= 0 and g3 in psTs:
                    ohts[g3] = copy_stage(psTs.pop(g3), g3)
                nxt_head = head(g) if g < n_groups else None
                if g - 2 >= 0 and (g - 2) in exps:
                    psTs[g - 2] = tr_stage(g - 2, exps.pop(g - 2))
                g4 = g - 4
                if g4 >= 0 and g4 in ohts:
                    po = gather_stage(g4, ohts.pop(g4), half=g4 % 2)
                    if po is not None:
                        pos[g4 // 2] = po
                if p_head is not None:
                    exps[g - 1] = exp_stage(g - 1, *p_head)
                g6 = g - 6
                if g6 >= 0 and g6 % 2 == 1 and (g6 // 2) in pos:
                    tail(g6, pos.pop(g6 // 2))
                p_head = nxt_head

    _legalize_waits(nc)
    nc.finalize()
    return nc


def _legalize_waits(nc):
    """This container's walrus accepts only ONE sync wait per engine
    instruction (setupSyncWait: 'Too many sync wait commands'). Tile emits
    multi-wait instructions (and an 11-wait tail drain). Split: keep one
    wait on the instruction, hoist the rest onto single-wait Drain ops
    inserted just before it on the same engine (engine order preserved =>
    semantics preserved). DMA copies are left alone (ring descriptors
    accept multiple waits)."""
    n_split = 0
    for f in nc.m.functions:
        for b in f.blocks:
            out = []
            for inst in b.instructions:
                si = inst.sync_info
                if si is not None and len(si.on_wait) > 1:
                    waits = list(si.on_wait)
                    for j, w in enumerate(waits[:-1]):
                        out.append(
                            mybir.InstDrain(
                                name=f"{inst.name}-w{j}",
                                engine=inst.engine,
                                ins=[],
                                outs=[],
                                sync_info=mybir.SyncInfo(
                                    on_wait=[w], on_update=[]
                                ),
                            )
                        )
                    inst.sync_info = mybir.SyncInfo(
                        on_wait=[waits[-1]], on_update=list(si.on_update)
                    )
                    n_split += 1
                out.append(inst)
            b.instructions = out
    return n_split


_NC = None


def _get_nc():
    global _NC
    if _NC is None:
        _NC = _build()
    return _NC


def _host_prep(x, memory):
    import ml_dtypes
    f8 = ml_dtypes.float8_e4m3

    memn = memory / np.maximum(
        np.sqrt((memory * memory).sum(axis=1, keepdims=True)), 1e-12
    )
    # Scale the normalized memory by BETA so PSUM holds beta*s directly
    # (exp sharpness K_eff = beta ~ 1e5). Cap keeps fp16 mh finite.
    beta = min(1e5, 55000.0 / max(float(np.abs(memn).max()), 1e-6))
    mnt = np.ascontiguousarray(memn.T).astype(np.float32) * beta   # [C, M]
    mh = mnt.astype(np.float16)
    ml = (mnt - mh.astype(np.float32)).astype(np.float32)

    # DR rhs slot tables (all exact power-of-2 shifts of fp8 encodings)
    mhf = mh.astype(np.float32)
    m8 = np.zeros((C, 4, M), dtype=f8)
    m8[:, 0, :] = (mhf / SIG).astype(f8)             # mhA
    m8[:, 1, :] = (mhf / (SIG * 16.0)).astype(f8)    # mhB
    m8[:, 2, :] = ml.astype(f8)                      # mlA
    m8[:, 3, :] = (ml / 16.0).astype(f8)             # mlB

    xh = x.astype(np.float16)
    xl = (x - xh.astype(np.float32)).astype(np.float32)
    xl1 = (xl * SIG).astype(f8)
    xl2 = ((xl * SIG - xl1.astype(np.float32)) * 16.0).astype(f8)
    xh8a = xh.astype(np.float32).astype(f8)
    xh8b = ((xh.astype(np.float32) - xh8a.astype(np.float32)) * 16.0).astype(f8)
    x8 = np.stack([xl1, xl2, xh8a, xh8b], axis=2)    # [B, C, 4, HW...]

    gh8 = memory.astype(f8)
    gl8 = (memory - gh8.astype(np.float32)).astype(f8)
    g8 = np.zeros((TILE, KCH * 2 * C), dtype=f8)
    for k in range(KCH):
        base = k * 2 * C
        g8[:, base : base + C] = gh8[k * TILE : (k + 1) * TILE, :]
        g8[:, base + C : base + 2 * C] = gl8[k * TILE : (k + 1) * TILE, :]

    return xh, x8, mh, m8, g8


def kernel(x, memory):
    x = np.asarray(x, dtype=np.float32)
    memory = np.asarray(memory, dtype=np.float32)
    nc = _get_nc()
    xf = x.reshape(B, C, N)
    xh, x8, mh16, m8, g8 = _host_prep(xf, memory)
    idu = np.eye(TILE, dtype=np.float16)

    in_maps = []
    for c in range(NCORES):
        in_maps.append({
            "xh": np.ascontiguousarray(xh[c * BPC : (c + 1) * BPC]),
            "x8": np.ascontiguousarray(x8[c * BPC : (c + 1) * BPC]),
            "mh16": mh16, "m8": m8, "g8": g8, "idu": idu,
        })

    res = run_bass_kernel_spmd(nc, in_maps, core_ids=list(range(NCORES)))
    outs = [
        r["out"].astype(np.float32).reshape(BPC, C, H, W) for r in res.results
    ]
    return np.concatenate(outs, axis=0)


# revision 24
# speedup vs baseline: 1.3038x; 1.0012x over previous
"""Trainium2 Bass kernel for nn_HardMemory (retrieval_knn).

For each spatial token (B*H*W tokens, C=128 channels), find the memory row
(of M=512) with max cosine similarity and replace the token's channel vector
with that raw memory row.

Algebraic simplification: argmax_m cos(x, mem_m) = argmax_m (x . mem_n_m)
where mem_n is the l2-normalized memory -- normalizing x is a positive
per-token scale and cannot change the argmax, so it is skipped.

Scores (PSUM fp32, beta-scaled): s = xh.mh16 (one fp16 matmul) plus two fp8
DoubleRow matmuls carrying the precision cross-terms at half cost:
  DR-A: (xl1, xl2).(mhA, mhB)   ~= xl.mh      (xl = fp16 residual of x,
        split in two e4m3 terms pre-scaled by 2^16; mh slots down-scaled)
  DR-B: (xh8a, xh8b).(mlA, mlB) ~= xh.ml      (ml = fp16 residual of the
        beta-scaled memory; xh split in two e4m3 terms)
Each DoubleRow sums TWO independent 128-deep products at 0.5 cycles/row, so
per-tile score cost is 512+256+256 = 1024 PE cycles vs 1536 for the 3-term
fp16 scheme. Dropped terms are O(2^-15) of score scale; measured ~10 argmax
flips over 131072 tokens, well under the 2e-2 rel-err gate.

One-hot: exp(s - max) on ACT (bias = negated reduce_max, scale=1), written
as e4m3 bytes at stride 2 into a SHARED fp16 tile: tile j of the group owns
byte plane j. One fp16 128x128 transpose then moves BOTH tiles' one-hot
bytes at once (one-hot bytes are <= 0x38 so the fp16 lanes are always
finite positive and the identity-matmul transpose is exact). This halves
both the PE transpose cost and the PSUM->SBUF copy volume vs per-tile fp8
transposes.

The gather reconstructs memory rows from a 2-term fp8 (e4m3 hi+lo) split
via fp8 DoubleRow one-hot matmuls reading stride-2 byte views of the
transposed pack: per tile 4 DR matmuls (2 chunk-pairs x hi/lo) at 0.5
cycles/row. Output fp32 PSUM -> fp16 SBUF copy -> DMA, stored fp16 and
upcast on the host (fp8 recon err ~1e-3 rel, far under the gate).

Hard constraints that shape the engine assignment (walrus-verified):
  - GPSIMD/Pool cannot access PSUM at all -> every PSUM-side op (reduce,
    exp, one-hot copies, out-copies) must share DVE + ACT only.
  - Free-axis reduce_max is DVE-only and has NO 2x perf mode (1 elem/cyc
    regardless of dtype); an engine may read only ONE non-scalar operand
    from PSUM (so no tensor_tensor max-of-halves trick on PSUM scores).
  - DMA cannot read PSUM, and matmul output must be fp32 PSUM, so the
    gather out-copy is unavoidable engine work.
Engine busy per group (2 tiles, 256 tokens), 64 groups per core:
  PE  : scores 853ns + transposes 213ns + gather 213ns   = 1280ns
  DVE : 2 per-tile reduce_max 658ns + ~3/4 ohT copies    = ~1610ns
  ACT : 2 exps 612ns + tail copy 306ns + ~1/4 ohT copies = ~1680ns
DVE/ACT are the bottleneck pair (~90% busy in steady state of the
~122us span); the remainder is cross-engine semaphore latency and
pipeline fill/drain.

Pipelining: per-TILE PSUM score tiles (1 bank each, 4 bufs) make the
buffer-release semaphore fire per tile, so head(g+2) tile j waits only on
exp(g) tile j (612ns earlier than the group's second exp). With a shared
2-tile score tile the release counts BOTH exp reads and the recycle chain
head->reduce->exp->head serializes to a ~2.4us period (measured); per-tile
tiles bring the period down to the DVE/ACT busy floor.

Sharding: data-parallel over batch, 4 batches per core, memory replicated.
Input DMAs are sliced and spread across groups so the 625ns HWDGE
descriptor setups never serialize against compute.
"""

import numpy as np

import concourse.bass as bass
import concourse.mybir as mybir
from concourse.tile import TileContext
from concourse.bass_utils import run_bass_kernel_spmd

F32 = mybir.dt.float32
F16 = mybir.dt.float16
F8 = mybir.dt.float8e4
AF = mybir.ActivationFunctionType

B, C, H, W = 32, 128, 64, 64
N = H * W              # 4096 tokens per batch
M = 512                # memory rows
NCORES = 8
BPC = B // NCORES      # batches per core
TOK = BPC * N          # tokens per core
TILE = 128             # tokens per tile
GRP = 2                # tiles per PSUM score group
LOAD = 4096            # tokens per input DMA region (one full batch image)
STORE = 512            # tokens per output DMA chunk (2 groups)
KCH = M // TILE        # 4 gather chunks
SIG = 65536.0          # xl pre-scale for the fp8 split (exact power of 2)


def _build():
    nc = bass.Bass(trn_type="TRN2")

    xh_in = nc.dram_tensor("xh", [BPC, C, N], F16, kind="ExternalInput")
    # fp8 slots: 0=xl1 1=xl2 (DR-A lhsT pair), 2=xh8a 3=xh8b (DR-B pair)
    x8_in = nc.dram_tensor("x8", [BPC, C, 4, N], F8, kind="ExternalInput")
    mh_in = nc.dram_tensor("mh16", [C, M], F16, kind="ExternalInput")
    # fp8 slots: 0=mhA 1=mhB (DR-A rhs pair), 2=mlA 3=mlB (DR-B rhs pair)
    m8_in = nc.dram_tensor("m8", [C, 4, M], F8, kind="ExternalInput")
    # raw memory rows fp8 hi/lo [TILE, KCH, 2, C] packed
    g8_in = nc.dram_tensor("g8", [TILE, KCH * 2 * C], F8, kind="ExternalInput")
    idu_in = nc.dram_tensor("idu", [TILE, TILE], F16, kind="ExternalInput")
    out_d = nc.dram_tensor("out", [BPC, C, N], F16, kind="ExternalOutput")

    with TileContext(nc) as tc:
        with (
            tc.tile_pool(name="const", bufs=1) as cpool,
            tc.tile_pool(name="xin", bufs=4) as xpool,
            tc.tile_pool(name="oh", bufs=8) as ohpool,
            tc.tile_pool(name="oht", bufs=2) as ohtpool,
            tc.tile_pool(name="osb", bufs=4) as opool,
            tc.tile_pool(name="small", bufs=12) as spool,
            tc.tile_pool(name="ps_s", bufs=4, space="PSUM") as ps_s,
            tc.tile_pool(name="ps_t", bufs=2, space="PSUM") as ps_t,
            tc.tile_pool(name="ps_o", bufs=2, space="PSUM") as ps_o,
        ):
            n_groups = TOK // (TILE * GRP)
            grp_per_batch = N // (TILE * GRP)
            loaded = {}

            def load_slice(b, s0, s1):
                nb = 8 if s1 - s0 == 1024 else 1
                xh_sb = xpool.tile(
                    [C, s1 - s0], F16, tag=f"xh{s1 - s0}", bufs=nb
                )
                nc.sync.dma_start(out=xh_sb, in_=xh_in[b, :, s0:s1])
                x8_sb = xpool.tile(
                    [C, 4, s1 - s0], F8, tag=f"x8{s1 - s0}", bufs=nb
                )
                nc.sync.dma_start(out=x8_sb, in_=x8_in[b, :, :, s0:s1])
                loaded.setdefault(b, []).append((s0, s1, xh_sb, x8_sb))

            def xslice(b, o, size):
                for s0, s1, xh_sb, x8_sb in loaded[b]:
                    if s0 <= o and o + size <= s1:
                        return (xh_sb[:, o - s0 : o - s0 + size],
                                x8_sb[:, :, o - s0 : o - s0 + size])
                raise AssertionError((b, o, size))

            # Prefetch schedule: batches 1..3 load one 1024-token slice per
            # group, starting 12 groups before the batch is needed.
            load_plan = {}
            for b in range(1, BPC):
                for si in range(4):
                    load_plan.setdefault(
                        b * grp_per_batch - 12 + 2 * si, []
                    ).append((b, si * 1024, (si + 1) * 1024))

            mh16 = cpool.tile([C, M], F16)
            nc.sync.dma_start(out=mh16, in_=mh_in[:])
            m8 = cpool.tile([C, 4, M], F8)
            nc.sync.dma_start(out=m8, in_=m8_in[:])
            # batch 0: small leading slices so PE starts early
            for s0, s1 in ((0, 256), (256, 1024), (1024, 2048), (2048, 3072),
                           (3072, N)):
                load_slice(0, s0, s1)
            g8 = cpool.tile([TILE, KCH * 2 * C], F8)
            nc.sync.dma_start(out=g8, in_=g8_in[:])
            g8v = g8.rearrange("p (k h c) -> p k h c", k=KCH, h=2)
            idu = cpool.tile([TILE, TILE], F16)
            nc.sync.dma_start(out=idu, in_=idu_in[:])

            DR = mybir.MatmulPerfMode.DoubleRow

            def head(g):
                """Score matmuls for group g + per-tile negated max.
                Each tile gets its OWN 1-bank PSUM tile and reduce so the
                buffer-release semaphore fires per tile: head(g+2) tile j
                only waits on exp(g) tile j, keeping the PSUM-recycle cycle
                well under 2x the engine-busy period."""
                gtok0 = g * TILE * GRP
                b = gtok0 // N
                for lb, s0, s1 in load_plan.get(g, ()):
                    load_slice(lb, s0, s1)
                psts = []
                nbmxs = []
                for j in range(GRP):
                    tok0 = gtok0 + j * TILE
                    o = tok0 % LOAD
                    xht, x8t = xslice(b, o, TILE)
                    ps = ps_s.tile([TILE, M], F32, tag="pst")
                    nc.tensor.matmul(out=ps, lhsT=xht, rhs=mh16,
                                     start=True, stop=False)
                    nc.tensor.matmul(out=ps, lhsT=x8t[:, 0:2, :],
                                     rhs=m8[:, 0:2, :],
                                     start=False, stop=False, perf_mode=DR)
                    nc.tensor.matmul(out=ps, lhsT=x8t[:, 2:4, :],
                                     rhs=m8[:, 2:4, :],
                                     start=False, stop=True, perf_mode=DR)
                    nbmx = spool.tile([TILE, 1], F32, tag="nbmx")
                    nc.vector.reduce_max(
                        out=nbmx, in_=ps, axis=mybir.AxisListType.X,
                        negate=True,
                    )
                    psts.append(ps)
                    nbmxs.append(nbmx)
                return psts, nbmxs

            def exp_stage(g, psts, nbmxs):
                """Exp one-hot for group g (1 group late): both tiles write
                fp8 bytes into one shared fp16-typed pack tile (tile j owns
                byte plane j)."""
                ohp = ohpool.tile([TILE, M], F16)
                oh8 = ohp.bitcast(F8).rearrange("p (m two) -> p m two", two=2)
                for j in range(GRP):
                    nc.scalar.activation(
                        out=oh8[:, :, j], in_=psts[j], func=AF.Exp,
                        bias=nbmxs[j], scale=1.0,
                    )
                return ohp

            def tr_stage(g, ohp):
                """4 fp16 transposes move both byte planes at once (2 groups
                late) into a per-group 1-bank PSUM tile."""
                psT = ps_t.tile([TILE, KCH, TILE], F16, tag="psT")
                for k in range(KCH):
                    nc.tensor.transpose(
                        out=psT[:, k, :],
                        in_=ohp[:, k * TILE : (k + 1) * TILE],
                        identity=idu,
                    )
                return psT

            def copy_stage(psT, g):
                """Per-group copy (DVE 2x fp16 mode; every 8th on ACT to
                shave the DVE bottleneck) moves the transposed one-hots to
                SBUF, one iteration after the transposes. (GPSIMD cannot
                access PSUM, so DVE/ACT carry all copies.)"""
                ohT = ohtpool.tile([TILE, KCH, TILE], F16)
                if g % 4 == 0:
                    nc.scalar.activation(out=ohT, in_=psT, func=AF.Copy)
                else:
                    nc.vector.tensor_copy(ohT, psT)
                return ohT

            po_cur = [None]

            def gather_stage(g, ohT, half):
                """fp8 DoubleRow gather (3 groups late): per tile 4 DR
                matmuls (chunk-pair x hi/lo). Pairs of groups share one
                [C, 512] fp32 PSUM bank."""
                if g % 2 == 0:
                    po_tile = ps_o.tile([C, STORE], F32, tag="po")
                    po_cur[0] = po_tile
                po = po_cur[0]
                ohT8 = ohT.bitcast(F8).rearrange(
                    "p k (t two) -> p k t two", two=2
                )
                for j in range(GRP):
                    col0 = (g % 2) * GRP * TILE + j * TILE
                    n_mm = 0
                    for pr in range(KCH // 2):
                        for h in range(2):
                            nc.tensor.matmul(
                                out=po[:, col0 : col0 + TILE],
                                lhsT=g8v[:, 2 * pr : 2 * pr + 2, h, :],
                                rhs=ohT8[:, 2 * pr : 2 * pr + 2, :, j],
                                start=(n_mm == 0), stop=(n_mm == 2 * KCH - 1),
                                perf_mode=DR,
                            )
                            n_mm += 1
                return po if g % 2 == 1 else None

            def tail(g, po):
                """Batched out-copy on ACT (5 groups late): [C, 512] fp32
                PSUM -> fp16 SBUF, then one DMA per STORE tokens."""
                gtok0 = g * TILE * GRP
                ob = opool.tile([C, STORE], F16, tag="ob")
                nc.scalar.activation(out=ob, in_=po, func=AF.Copy)
                b, n0 = divmod(gtok0 + GRP * TILE - STORE, N)
                nc.sync.dma_start(out=out_d[b, :, n0 : n0 + STORE], in_=ob)

            # Software pipeline, one iteration per score group g:
            #   PE : tr(g-2) | gather(g-5) | head(g)     (ready work first;
            #        head's buf wait is the in-order SEQ block point)
            #   DVE: ohT copy (pair (g-4)//2) | reduce(g)
            #   ACT: exps(g-1) | tail copy(g-6)
            # The head->reduce->exp->head PSUM recycle is the critical cycle;
            # per-TILE subtile deps (exp tile j releases the score buf slice
            # for head(g+2) tile j) keep it under 2x the engine-busy period.
            p_head = None
            exps = {}
            psTs = {}
            ohts = {}
            pos = {}
            for g in range(n_groups + 8):
                # Per-iteration stages: head(g) | exp(g-1) | tr(g-2) |
                # copy(g-3) | gather(g-4) | tail(g-6, per pair). Every
                # cross-engine dependency is >=1 iteration old; the score-buf
                # recycle (head(g+2) tile j <- exp(g) tile j) is the
                # critical cycle, kept short by per-tile PSUM tiles.
                nxt_head = head(g) if g < n_groups else None
                g3 = g - 3
                if g3 >= 0 and g3 in psTs:
                    ohts[g3] = copy_stage(psTs.pop(g3), g3)
                if g - 2 >= 0 and (g - 2) in exps:
                    psTs[g - 2] = tr_stage(g - 2, exps.pop(g - 2))
                g4 = g - 4
                if g4 >= 0 and g4 in ohts:
                    po = gather_stage(g4, ohts.pop(g4), half=g4 % 2)
                    if po is not None:
                        pos[g4 // 2] = po
                if p_head is not None:
                    exps[g - 1] = exp_stage(g - 1, *p_head)
                g6 = g - 6
                if g6 >= 0 and g6 % 2 == 1 and (g6 // 2) in pos:
                    tail(g6, pos.pop(g6 // 2))
                p_head = nxt_head

    _legalize_waits(nc)
    nc.finalize()
    return nc


def _legalize_waits(nc):
    """This container's walrus accepts only ONE sync wait per engine
    instruction (setupSyncWait: 'Too many sync wait commands'). Tile emits
    multi-wait instructions (and an 11-wait tail drain). Split: keep one
    wait on the instruction, hoist the rest onto single-wait Drain ops
    inserted just before it on the same engine (engine order preserved =>
    semantics preserved). DMA copies are left alone (ring descriptors
    accept multiple waits)."""
    n_split = 0
    for f in nc.m.functions:
        for b in f.blocks:
            out = []
            for inst in b.instructions:
                si = inst.sync_info
                if si is not None and len(si.on_wait) > 1:
                    waits = list(si.on_wait)
                    for j, w in enumerate(waits[:-1]):
                        out.append(
                            mybir.InstDrain(
                                name=f"{inst.name}-w{j}",
                                engine=inst.engine,
                                ins=[],
                                outs=[],
                                sync_info=mybir.SyncInfo(
                                    on_wait=[w], on_update=[]
                                ),
                            )
                        )
                    inst.sync_info = mybir.SyncInfo(
                        on_wait=[waits[-1]], on_update=list(si.on_update)
                    )
                    n_split += 1
                out.append(inst)
            b.instructions = out
    return n_split


_NC = None


def _get_nc():
    global _NC
    if _NC is None:
        _NC = _build()
    return _NC


def _host_prep(x, memory):
    import ml_dtypes
    f8 = ml_dtypes.float8_e4m3

    memn = memory / np.maximum(
        np.sqrt((memory * memory).sum(axis=1, keepdims=True)), 1e-12
    )
    # Scale the normalized memory by BETA so PSUM holds beta*s directly
    # (exp sharpness K_eff = beta ~ 1e5). Cap keeps fp16 mh finite.
    beta = min(1e5, 55000.0 / max(float(np.abs(memn).max()), 1e-6))
    mnt = np.ascontiguousarray(memn.T).astype(np.float32) * beta   # [C, M]
    mh = mnt.astype(np.float16)
    ml = (mnt - mh.astype(np.float32)).astype(np.float32)

    # DR rhs slot tables (all exact power-of-2 shifts of fp8 encodings)
    mhf = mh.astype(np.float32)
    m8 = np.zeros((C, 4, M), dtype=f8)
    m8[:, 0, :] = (mhf / SIG).astype(f8)             # mhA
    m8[:, 1, :] = (mhf / (SIG * 16.0)).astype(f8)    # mhB
    m8[:, 2, :] = ml.astype(f8)                      # mlA
    m8[:, 3, :] = (ml / 16.0).astype(f8)             # mlB

    xh = x.astype(np.float16)
    xl = (x - xh.astype(np.float32)).astype(np.float32)
    xl1 = (xl * SIG).astype(f8)
    xl2 = ((xl * SIG - xl1.astype(np.float32)) * 16.0).astype(f8)
    xh8a = xh.astype(np.float32).astype(f8)
    xh8b = ((xh.astype(np.float32) - xh8a.astype(np.float32)) * 16.0).astype(f8)
    x8 = np.stack([xl1, xl2, xh8a, xh8b], axis=2)    # [B, C, 4, HW...]

    gh8 = memory.astype(f8)
    gl8 = (memory - gh8.astype(np.float32)).astype(f8)
    g8 = np.zeros((TILE, KCH * 2 * C), dtype=f8)
    for k in range(KCH):
        base = k * 2 * C
        g8[:, base : base + C] = gh8[k * TILE : (k + 1) * TILE, :]
        g8[:, base + C : base + 2 * C] = gl8[k * TILE : (k + 1) * TILE, :]

    return xh, x8, mh, m8, g8


def kernel(x, memory):
    x = np.asarray(x, dtype=np.float32)
    memory = np.asarray(memory, dtype=np.float32)
    nc = _get_nc()
    xf = x.reshape(B, C, N)
    xh, x8, mh16, m8, g8 = _host_prep(xf, memory)
    idu = np.eye(TILE, dtype=np.float16)

    in_maps = []
    for c in range(NCORES):
        in_maps.append({
            "xh": np.ascontiguousarray(xh[c * BPC : (c + 1) * BPC]),
            "x8": np.ascontiguousarray(x8[c * BPC : (c + 1) * BPC]),
            "mh16": mh16, "m8": m8, "g8": g8, "idu": idu,
        })

    res = run_bass_kernel_spmd(nc, in_maps, core_ids=list(range(NCORES)))
    outs = [
        r["out"].astype(np.float32).reshape(BPC, C, H, W) for r in res.results
    ]
    return np.concatenate(outs, axis=0)


# revision 25
# speedup vs baseline: 1.3074x; 1.0028x over previous
"""Trainium2 Bass kernel for nn_HardMemory (retrieval_knn).

For each spatial token (B*H*W tokens, C=128 channels), find the memory row
(of M=512) with max cosine similarity and replace the token's channel vector
with that raw memory row.

Algebraic simplification: argmax_m cos(x, mem_m) = argmax_m (x . mem_n_m)
where mem_n is the l2-normalized memory -- normalizing x is a positive
per-token scale and cannot change the argmax, so it is skipped.

Scores (PSUM fp32, beta-scaled): s = xh.mh16 (one fp16 matmul) plus two fp8
DoubleRow matmuls carrying the precision cross-terms at half cost:
  DR-A: (xl1, xl2).(mhA, mhB)   ~= xl.mh      (xl = fp16 residual of x,
        split in two e4m3 terms pre-scaled by 2^16; mh slots down-scaled)
  DR-B: (xh8a, xh8b).(mlA, mlB) ~= xh.ml      (ml = fp16 residual of the
        beta-scaled memory; xh split in two e4m3 terms)
Each DoubleRow sums TWO independent 128-deep products at 0.5 cycles/row, so
per-tile score cost is 512+256+256 = 1024 PE cycles vs 1536 for the 3-term
fp16 scheme. Dropped terms are O(2^-15) of score scale; measured ~10 argmax
flips over 131072 tokens, well under the 2e-2 rel-err gate.

One-hot: exp(s - max) on ACT (bias = negated reduce_max, scale=1), written
as e4m3 bytes at stride 2 into a SHARED fp16 tile: tile j of the group owns
byte plane j. One fp16 128x128 transpose then moves BOTH tiles' one-hot
bytes at once (one-hot bytes are <= 0x38 so the fp16 lanes are always
finite positive and the identity-matmul transpose is exact). This halves
both the PE transpose cost and the PSUM->SBUF copy volume vs per-tile fp8
transposes.

The gather reconstructs memory rows from a 2-term fp8 (e4m3 hi+lo) split
via fp8 DoubleRow one-hot matmuls reading stride-2 byte views of the
transposed pack: per tile 4 DR matmuls (2 chunk-pairs x hi/lo) at 0.5
cycles/row. Output fp32 PSUM -> fp16 SBUF copy -> DMA, stored fp16 and
upcast on the host (fp8 recon err ~1e-3 rel, far under the gate).

Hard constraints that shape the engine assignment (walrus-verified):
  - GPSIMD/Pool cannot access PSUM at all -> every PSUM-side op (reduce,
    exp, one-hot copies, out-copies) must share DVE + ACT only.
  - Free-axis reduce_max is DVE-only and has NO 2x perf mode (1 elem/cyc
    regardless of dtype); an engine may read only ONE non-scalar operand
    from PSUM (so no tensor_tensor max-of-halves trick on PSUM scores).
  - DMA cannot read PSUM, and matmul output must be fp32 PSUM, so the
    gather out-copy is unavoidable engine work.
Engine busy per group (2 tiles, 256 tokens), 64 groups per core:
  PE  : scores 853ns + transposes 213ns + gather 213ns   = 1280ns
  DVE : 2 per-tile reduce_max 658ns + ~3/4 ohT copies    = ~1610ns
  ACT : 2 exps 612ns + tail copy 306ns + ~1/4 ohT copies = ~1680ns
DVE/ACT are the bottleneck pair (~90% busy in steady state of the
~122us span); the remainder is cross-engine semaphore latency and
pipeline fill/drain.

Pipelining: per-TILE PSUM score tiles (1 bank each, 4 bufs) make the
buffer-release semaphore fire per tile, so head(g+2) tile j waits only on
exp(g) tile j (612ns earlier than the group's second exp). With a shared
2-tile score tile the release counts BOTH exp reads and the recycle chain
head->reduce->exp->head serializes to a ~2.4us period (measured); per-tile
tiles bring the period down to the DVE/ACT busy floor.

Sharding: data-parallel over batch, 4 batches per core, memory replicated.
Input DMAs are sliced and spread across groups so the 625ns HWDGE
descriptor setups never serialize against compute.
"""

import numpy as np

import concourse.bass as bass
import concourse.mybir as mybir
from concourse.tile import TileContext
from concourse.bass_utils import run_bass_kernel_spmd

F32 = mybir.dt.float32
F16 = mybir.dt.float16
F8 = mybir.dt.float8e4
AF = mybir.ActivationFunctionType

B, C, H, W = 32, 128, 64, 64
N = H * W              # 4096 tokens per batch
M = 512                # memory rows
NCORES = 8
BPC = B // NCORES      # batches per core
TOK = BPC * N          # tokens per core
TILE = 128             # tokens per tile
GRP = 2                # tiles per PSUM score group
LOAD = 4096            # tokens per input DMA region (one full batch image)
STORE = 512            # tokens per output DMA chunk (2 groups)
KCH = M // TILE        # 4 gather chunks
SIG = 65536.0          # xl pre-scale for the fp8 split (exact power of 2)


def _build():
    nc = bass.Bass(trn_type="TRN2")

    xh_in = nc.dram_tensor("xh", [BPC, C, N], F16, kind="ExternalInput")
    # fp8 slots: 0=xl1 1=xl2 (DR-A lhsT pair), 2=xh8a 3=xh8b (DR-B pair)
    x8_in = nc.dram_tensor("x8", [BPC, C, 4, N], F8, kind="ExternalInput")
    mh_in = nc.dram_tensor("mh16", [C, M], F16, kind="ExternalInput")
    # fp8 slots: 0=mhA 1=mhB (DR-A rhs pair), 2=mlA 3=mlB (DR-B rhs pair)
    m8_in = nc.dram_tensor("m8", [C, 4, M], F8, kind="ExternalInput")
    # raw memory rows fp8 hi/lo [TILE, KCH, 2, C] packed
    g8_in = nc.dram_tensor("g8", [TILE, KCH * 2 * C], F8, kind="ExternalInput")
    idu_in = nc.dram_tensor("idu", [TILE, TILE], F16, kind="ExternalInput")
    out_d = nc.dram_tensor("out", [BPC, C, N], F16, kind="ExternalOutput")

    with TileContext(nc) as tc:
        with (
            tc.tile_pool(name="const", bufs=1) as cpool,
            tc.tile_pool(name="xin", bufs=4) as xpool,
            tc.tile_pool(name="oh", bufs=8) as ohpool,
            tc.tile_pool(name="oht", bufs=2) as ohtpool,
            tc.tile_pool(name="osb", bufs=6) as opool,
            tc.tile_pool(name="small", bufs=16) as spool,
            tc.tile_pool(name="ps_s", bufs=4, space="PSUM") as ps_s,
            tc.tile_pool(name="ps_t", bufs=2, space="PSUM") as ps_t,
            tc.tile_pool(name="ps_o", bufs=2, space="PSUM") as ps_o,
        ):
            n_groups = TOK // (TILE * GRP)
            grp_per_batch = N // (TILE * GRP)
            loaded = {}

            def load_slice(b, s0, s1):
                nb = 8 if s1 - s0 == 1024 else 1
                xh_sb = xpool.tile(
                    [C, s1 - s0], F16, tag=f"xh{s1 - s0}", bufs=nb
                )
                nc.sync.dma_start(out=xh_sb, in_=xh_in[b, :, s0:s1])
                x8_sb = xpool.tile(
                    [C, 4, s1 - s0], F8, tag=f"x8{s1 - s0}", bufs=nb
                )
                nc.sync.dma_start(out=x8_sb, in_=x8_in[b, :, :, s0:s1])
                loaded.setdefault(b, []).append((s0, s1, xh_sb, x8_sb))

            def xslice(b, o, size):
                for s0, s1, xh_sb, x8_sb in loaded[b]:
                    if s0 <= o and o + size <= s1:
                        return (xh_sb[:, o - s0 : o - s0 + size],
                                x8_sb[:, :, o - s0 : o - s0 + size])
                raise AssertionError((b, o, size))

            # Prefetch schedule: batches 1..3 load one 1024-token slice per
            # group, starting 12 groups before the batch is needed.
            load_plan = {}
            for b in range(1, BPC):
                for si in range(4):
                    load_plan.setdefault(
                        b * grp_per_batch - 12 + 2 * si, []
                    ).append((b, si * 1024, (si + 1) * 1024))

            mh16 = cpool.tile([C, M], F16)
            nc.sync.dma_start(out=mh16, in_=mh_in[:])
            m8 = cpool.tile([C, 4, M], F8)
            nc.sync.dma_start(out=m8, in_=m8_in[:])
            # batch 0: small leading slices so PE starts early
            for s0, s1 in ((0, 256), (256, 1024), (1024, 2048), (2048, 3072),
                           (3072, N)):
                load_slice(0, s0, s1)
            g8 = cpool.tile([TILE, KCH * 2 * C], F8)
            nc.sync.dma_start(out=g8, in_=g8_in[:])
            g8v = g8.rearrange("p (k h c) -> p k h c", k=KCH, h=2)
            idu = cpool.tile([TILE, TILE], F16)
            nc.sync.dma_start(out=idu, in_=idu_in[:])

            DR = mybir.MatmulPerfMode.DoubleRow

            def head(g):
                """Score matmuls for group g + per-tile negated max.
                Each tile gets its OWN 1-bank PSUM tile and reduce so the
                buffer-release semaphore fires per tile: head(g+2) tile j
                only waits on exp(g) tile j, keeping the PSUM-recycle cycle
                well under 2x the engine-busy period."""
                gtok0 = g * TILE * GRP
                b = gtok0 // N
                for lb, s0, s1 in load_plan.get(g, ()):
                    load_slice(lb, s0, s1)
                psts = []
                nbmxs = []
                for j in range(GRP):
                    tok0 = gtok0 + j * TILE
                    o = tok0 % LOAD
                    xht, x8t = xslice(b, o, TILE)
                    ps = ps_s.tile([TILE, M], F32, tag="pst")
                    nc.tensor.matmul(out=ps, lhsT=xht, rhs=mh16,
                                     start=True, stop=False)
                    nc.tensor.matmul(out=ps, lhsT=x8t[:, 0:2, :],
                                     rhs=m8[:, 0:2, :],
                                     start=False, stop=False, perf_mode=DR)
                    nc.tensor.matmul(out=ps, lhsT=x8t[:, 2:4, :],
                                     rhs=m8[:, 2:4, :],
                                     start=False, stop=True, perf_mode=DR)
                    nbmx = spool.tile([TILE, 1], F32, tag="nbmx")
                    nc.vector.reduce_max(
                        out=nbmx, in_=ps, axis=mybir.AxisListType.X,
                        negate=True,
                    )
                    psts.append(ps)
                    nbmxs.append(nbmx)
                return psts, nbmxs

            def exp_stage(g, psts, nbmxs):
                """Exp one-hot for group g (1 group late): both tiles write
                fp8 bytes into one shared fp16-typed pack tile (tile j owns
                byte plane j)."""
                ohp = ohpool.tile([TILE, M], F16)
                oh8 = ohp.bitcast(F8).rearrange("p (m two) -> p m two", two=2)
                for j in range(GRP):
                    nc.scalar.activation(
                        out=oh8[:, :, j], in_=psts[j], func=AF.Exp,
                        bias=nbmxs[j], scale=1.0,
                    )
                return ohp

            def tr_stage(g, ohp):
                """4 fp16 transposes move both byte planes at once (2 groups
                late) into a per-group 1-bank PSUM tile."""
                psT = ps_t.tile([TILE, KCH, TILE], F16, tag="psT")
                for k in range(KCH):
                    nc.tensor.transpose(
                        out=psT[:, k, :],
                        in_=ohp[:, k * TILE : (k + 1) * TILE],
                        identity=idu,
                    )
                return psT

            def copy_stage(psT, g):
                """Per-group copy (DVE 2x fp16 mode; every 8th on ACT to
                shave the DVE bottleneck) moves the transposed one-hots to
                SBUF, one iteration after the transposes. (GPSIMD cannot
                access PSUM, so DVE/ACT carry all copies.)"""
                ohT = ohtpool.tile([TILE, KCH, TILE], F16)
                if g % 4 == 0:
                    nc.scalar.activation(out=ohT, in_=psT, func=AF.Copy)
                else:
                    nc.vector.tensor_copy(ohT, psT)
                return ohT

            po_cur = [None]

            def gather_stage(g, ohT, half):
                """fp8 DoubleRow gather (3 groups late): per tile 4 DR
                matmuls (chunk-pair x hi/lo). Pairs of groups share one
                [C, 512] fp32 PSUM bank."""
                if g % 2 == 0:
                    po_tile = ps_o.tile([C, STORE], F32, tag="po")
                    po_cur[0] = po_tile
                po = po_cur[0]
                ohT8 = ohT.bitcast(F8).rearrange(
                    "p k (t two) -> p k t two", two=2
                )
                for j in range(GRP):
                    col0 = (g % 2) * GRP * TILE + j * TILE
                    n_mm = 0
                    for pr in range(KCH // 2):
                        for h in range(2):
                            nc.tensor.matmul(
                                out=po[:, col0 : col0 + TILE],
                                lhsT=g8v[:, 2 * pr : 2 * pr + 2, h, :],
                                rhs=ohT8[:, 2 * pr : 2 * pr + 2, :, j],
                                start=(n_mm == 0), stop=(n_mm == 2 * KCH - 1),
                                perf_mode=DR,
                            )
                            n_mm += 1
                return po if g % 2 == 1 else None

            def tail(g, po):
                """Batched out-copy on ACT (5 groups late): [C, 512] fp32
                PSUM -> fp16 SBUF, then one DMA per STORE tokens."""
                gtok0 = g * TILE * GRP
                ob = opool.tile([C, STORE], F16, tag="ob")
                nc.scalar.activation(out=ob, in_=po, func=AF.Copy)
                b, n0 = divmod(gtok0 + GRP * TILE - STORE, N)
                nc.sync.dma_start(out=out_d[b, :, n0 : n0 + STORE], in_=ob)

            # Software pipeline, one iteration per score group g:
            #   PE : tr(g-2) | gather(g-5) | head(g)     (ready work first;
            #        head's buf wait is the in-order SEQ block point)
            #   DVE: ohT copy (pair (g-4)//2) | reduce(g)
            #   ACT: exps(g-1) | tail copy(g-6)
            # The head->reduce->exp->head PSUM recycle is the critical cycle;
            # per-TILE subtile deps (exp tile j releases the score buf slice
            # for head(g+2) tile j) keep it under 2x the engine-busy period.
            p_head = None
            exps = {}
            psTs = {}
            ohts = {}
            pos = {}
            for g in range(n_groups + 8):
                # Per-iteration stages: head(g) | exp(g-1) | tr(g-2) |
                # copy(g-3) | gather(g-4) | tail(g-6, per pair). Every
                # cross-engine dependency is >=1 iteration old; the score-buf
                # recycle (head(g+2) tile j <- exp(g) tile j) is the
                # critical cycle, kept short by per-tile PSUM tiles.
                nxt_head = head(g) if g < n_groups else None
                g3 = g - 3
                if g3 >= 0 and g3 in psTs:
                    ohts[g3] = copy_stage(psTs.pop(g3), g3)
                if g - 2 >= 0 and (g - 2) in exps:
                    psTs[g - 2] = tr_stage(g - 2, exps.pop(g - 2))
                g4 = g - 4
                if g4 >= 0 and g4 in ohts:
                    po = gather_stage(g4, ohts.pop(g4), half=g4 % 2)
                    if po is not None:
                        pos[g4 // 2] = po
                if p_head is not None:
                    exps[g - 1] = exp_stage(g - 1, *p_head)
                g6 = g - 6
                if g6 >= 0 and g6 % 2 == 1 and (g6 // 2) in pos:
                    tail(g6, pos.pop(g6 // 2))
                p_head = nxt_head

    _legalize_waits(nc)
    nc.finalize()
    return nc


def _legalize_waits(nc):
    """This container's walrus accepts only ONE sync wait per engine
    instruction (setupSyncWait: 'Too many sync wait commands'). Tile emits
    multi-wait instructions (and an 11-wait tail drain). Split: keep one
    wait on the instruction, hoist the rest onto single-wait Drain ops
    inserted just before it on the same engine (engine order preserved =>
    semantics preserved). DMA copies are left alone (ring descriptors
    accept multiple waits)."""
    n_split = 0
    for f in nc.m.functions:
        for b in f.blocks:
            out = []
            for inst in b.instructions:
                si = inst.sync_info
                if si is not None and len(si.on_wait) > 1:
                    waits = list(si.on_wait)
                    for j, w in enumerate(waits[:-1]):
                        out.append(
                            mybir.InstDrain(
                                name=f"{inst.name}-w{j}",
                                engine=inst.engine,
                                ins=[],
                                outs=[],
                                sync_info=mybir.SyncInfo(
                                    on_wait=[w], on_update=[]
                                ),
                            )
                        )
                    inst.sync_info = mybir.SyncInfo(
                        on_wait=[waits[-1]], on_update=list(si.on_update)
                    )
                    n_split += 1
                out.append(inst)
            b.instructions = out
    return n_split


_NC = None


def _get_nc():
    global _NC
    if _NC is None:
        _NC = _build()
    return _NC


def _host_prep(x, memory):
    import ml_dtypes
    f8 = ml_dtypes.float8_e4m3

    memn = memory / np.maximum(
        np.sqrt((memory * memory).sum(axis=1, keepdims=True)), 1e-12
    )
    # Scale the normalized memory by BETA so PSUM holds beta*s directly
    # (exp sharpness K_eff = beta ~ 1e5). Cap keeps fp16 mh finite.
    beta = min(1e5, 55000.0 / max(float(np.abs(memn).max()), 1e-6))
    mnt = np.ascontiguousarray(memn.T).astype(np.float32) * beta   # [C, M]
    mh = mnt.astype(np.float16)
    ml = (mnt - mh.astype(np.float32)).astype(np.float32)

    # DR rhs slot tables (all exact power-of-2 shifts of fp8 encodings)
    mhf = mh.astype(np.float32)
    m8 = np.zeros((C, 4, M), dtype=f8)
    m8[:, 0, :] = (mhf / SIG).astype(f8)             # mhA
    m8[:, 1, :] = (mhf / (SIG * 16.0)).astype(f8)    # mhB
    m8[:, 2, :] = ml.astype(f8)                      # mlA
    m8[:, 3, :] = (ml / 16.0).astype(f8)             # mlB

    xh = x.astype(np.float16)
    xl = (x - xh.astype(np.float32)).astype(np.float32)
    xl1 = (xl * SIG).astype(f8)
    xl2 = ((xl * SIG - xl1.astype(np.float32)) * 16.0).astype(f8)
    xh8a = xh.astype(np.float32).astype(f8)
    xh8b = ((xh.astype(np.float32) - xh8a.astype(np.float32)) * 16.0).astype(f8)
    x8 = np.stack([xl1, xl2, xh8a, xh8b], axis=2)    # [B, C, 4, HW...]

    gh8 = memory.astype(f8)
    gl8 = (memory - gh8.astype(np.float32)).astype(f8)
    g8 = np.zeros((TILE, KCH * 2 * C), dtype=f8)
    for k in range(KCH):
        base = k * 2 * C
        g8[:, base : base + C] = gh8[k * TILE : (k + 1) * TILE, :]
        g8[:, base + C : base + 2 * C] = gl8[k * TILE : (k + 1) * TILE, :]

    return xh, x8, mh, m8, g8


def kernel(x, memory):
    x = np.asarray(x, dtype=np.float32)
    memory = np.asarray(memory, dtype=np.float32)
    nc = _get_nc()
    xf = x.reshape(B, C, N)
    xh, x8, mh16, m8, g8 = _host_prep(xf, memory)
    idu = np.eye(TILE, dtype=np.float16)

    in_maps = []
    for c in range(NCORES):
        in_maps.append({
            "xh": np.ascontiguousarray(xh[c * BPC : (c + 1) * BPC]),
            "x8": np.ascontiguousarray(x8[c * BPC : (c + 1) * BPC]),
            "mh16": mh16, "m8": m8, "g8": g8, "idu": idu,
        })

    res = run_bass_kernel_spmd(nc, in_maps, core_ids=list(range(NCORES)))
    outs = [
        r["out"].astype(np.float32).reshape(BPC, C, H, W) for r in res.results
    ]
    return np.concatenate(outs, axis=0)


# revision 26
# speedup vs baseline: 1.3092x; 1.0014x over previous
"""Trainium2 Bass kernel for nn_HardMemory (retrieval_knn).

For each spatial token (B*H*W tokens, C=128 channels), find the memory row
(of M=512) with max cosine similarity and replace the token's channel vector
with that raw memory row.

Algebraic simplification: argmax_m cos(x, mem_m) = argmax_m (x . mem_n_m)
where mem_n is the l2-normalized memory -- normalizing x is a positive
per-token scale and cannot change the argmax, so it is skipped.

Scores (PSUM fp32, beta-scaled): s = xh.mh16 (one fp16 matmul) plus two fp8
DoubleRow matmuls carrying the precision cross-terms at half cost:
  DR-A: (xl1, xl2).(mhA, mhB)   ~= xl.mh      (xl = fp16 residual of x,
        split in two e4m3 terms pre-scaled by 2^16; mh slots down-scaled)
  DR-B: (xh8a, xh8b).(mlA, mlB) ~= xh.ml      (ml = fp16 residual of the
        beta-scaled memory; xh split in two e4m3 terms)
Each DoubleRow sums TWO independent 128-deep products at 0.5 cycles/row, so
per-tile score cost is 512+256+256 = 1024 PE cycles vs 1536 for the 3-term
fp16 scheme. Dropped terms are O(2^-15) of score scale; measured ~10 argmax
flips over 131072 tokens, well under the 2e-2 rel-err gate.

One-hot: exp(s - max) on ACT (bias = negated reduce_max, scale=1), written
as e4m3 bytes at stride 2 into a SHARED fp16 tile: tile j of the group owns
byte plane j. One fp16 128x128 transpose then moves BOTH tiles' one-hot
bytes at once (one-hot bytes are <= 0x38 so the fp16 lanes are always
finite positive and the identity-matmul transpose is exact). This halves
both the PE transpose cost and the PSUM->SBUF copy volume vs per-tile fp8
transposes.

The gather reconstructs memory rows from a 2-term fp8 (e4m3 hi+lo) split
via fp8 DoubleRow one-hot matmuls reading stride-2 byte views of the
transposed pack: per tile 4 DR matmuls (2 chunk-pairs x hi/lo) at 0.5
cycles/row. Output fp32 PSUM -> fp16 SBUF copy -> DMA, stored fp16 and
upcast on the host (fp8 recon err ~1e-3 rel, far under the gate).

Hard constraints that shape the engine assignment (walrus-verified):
  - GPSIMD/Pool cannot access PSUM at all -> every PSUM-side op (reduce,
    exp, one-hot copies, out-copies) must share DVE + ACT only.
  - Free-axis reduce_max is DVE-only and has NO 2x perf mode (1 elem/cyc
    regardless of dtype); an engine may read only ONE non-scalar operand
    from PSUM (so no tensor_tensor max-of-halves trick on PSUM scores).
  - DMA cannot read PSUM, and matmul output must be fp32 PSUM, so the
    gather out-copy is unavoidable engine work.
Engine busy per group (2 tiles, 256 tokens), 64 groups per core:
  PE  : scores 853ns + transposes 213ns + gather 213ns   = 1280ns
  DVE : 2 per-tile reduce_max 658ns + ~3/4 ohT copies    = ~1610ns
  ACT : 2 exps 612ns + tail copy 306ns + ~1/4 ohT copies = ~1680ns
DVE/ACT are the bottleneck pair (~90% busy in steady state of the
~122us span); the remainder is cross-engine semaphore latency and
pipeline fill/drain.

Pipelining: per-TILE PSUM score tiles (1 bank each, 4 bufs) make the
buffer-release semaphore fire per tile, so head(g+2) tile j waits only on
exp(g) tile j (612ns earlier than the group's second exp). With a shared
2-tile score tile the release counts BOTH exp reads and the recycle chain
head->reduce->exp->head serializes to a ~2.4us period (measured); per-tile
tiles bring the period down to the DVE/ACT busy floor.

Sharding: data-parallel over batch, 4 batches per core, memory replicated.
Input DMAs are sliced and spread across groups so the 625ns HWDGE
descriptor setups never serialize against compute.
"""

import numpy as np

import concourse.bass as bass
import concourse.mybir as mybir
from concourse.tile import TileContext
from concourse.bass_utils import run_bass_kernel_spmd

F32 = mybir.dt.float32
F16 = mybir.dt.float16
F8 = mybir.dt.float8e4
AF = mybir.ActivationFunctionType

B, C, H, W = 32, 128, 64, 64
N = H * W              # 4096 tokens per batch
M = 512                # memory rows
NCORES = 8
BPC = B // NCORES      # batches per core
TOK = BPC * N          # tokens per core
TILE = 128             # tokens per tile
GRP = 2                # tiles per PSUM score group
LOAD = 4096            # tokens per input DMA region (one full batch image)
STORE = 512            # tokens per output DMA chunk (2 groups)
KCH = M // TILE        # 4 gather chunks
SIG = 65536.0          # xl pre-scale for the fp8 split (exact power of 2)


def _build():
    nc = bass.Bass(trn_type="TRN2")

    xh_in = nc.dram_tensor("xh", [BPC, C, N], F16, kind="ExternalInput")
    # fp8 slots: 0=xl1 1=xl2 (DR-A lhsT pair), 2=xh8a 3=xh8b (DR-B pair)
    x8_in = nc.dram_tensor("x8", [BPC, C, 4, N], F8, kind="ExternalInput")
    mh_in = nc.dram_tensor("mh16", [C, M], F16, kind="ExternalInput")
    # fp8 slots: 0=mhA 1=mhB (DR-A rhs pair), 2=mlA 3=mlB (DR-B rhs pair)
    m8_in = nc.dram_tensor("m8", [C, 4, M], F8, kind="ExternalInput")
    # raw memory rows fp8 hi/lo [TILE, KCH, 2, C] packed
    g8_in = nc.dram_tensor("g8", [TILE, KCH * 2 * C], F8, kind="ExternalInput")
    idu_in = nc.dram_tensor("idu", [TILE, TILE], F16, kind="ExternalInput")
    out_d = nc.dram_tensor("out", [BPC, C, N], F16, kind="ExternalOutput")

    with TileContext(nc) as tc:
        with (
            tc.tile_pool(name="const", bufs=1) as cpool,
            tc.tile_pool(name="xin", bufs=4) as xpool,
            tc.tile_pool(name="oh", bufs=8) as ohpool,
            tc.tile_pool(name="oht", bufs=2) as ohtpool,
            tc.tile_pool(name="osb", bufs=12) as opool,
            tc.tile_pool(name="small", bufs=16) as spool,
            tc.tile_pool(name="ps_s", bufs=4, space="PSUM") as ps_s,
            tc.tile_pool(name="ps_t", bufs=2, space="PSUM") as ps_t,
            tc.tile_pool(name="ps_o", bufs=2, space="PSUM") as ps_o,
        ):
            n_groups = TOK // (TILE * GRP)
            grp_per_batch = N // (TILE * GRP)
            loaded = {}

            def load_slice(b, s0, s1):
                nb = 8 if s1 - s0 == 1024 else 1
                xh_sb = xpool.tile(
                    [C, s1 - s0], F16, tag=f"xh{s1 - s0}", bufs=nb
                )
                nc.sync.dma_start(out=xh_sb, in_=xh_in[b, :, s0:s1])
                x8_sb = xpool.tile(
                    [C, 4, s1 - s0], F8, tag=f"x8{s1 - s0}", bufs=nb
                )
                nc.sync.dma_start(out=x8_sb, in_=x8_in[b, :, :, s0:s1])
                loaded.setdefault(b, []).append((s0, s1, xh_sb, x8_sb))

            def xslice(b, o, size):
                for s0, s1, xh_sb, x8_sb in loaded[b]:
                    if s0 <= o and o + size <= s1:
                        return (xh_sb[:, o - s0 : o - s0 + size],
                                x8_sb[:, :, o - s0 : o - s0 + size])
                raise AssertionError((b, o, size))

            # Prefetch schedule: batches 1..3 load one 1024-token slice per
            # group, starting 12 groups before the batch is needed.
            load_plan = {}
            for b in range(1, BPC):
                for si in range(4):
                    load_plan.setdefault(
                        b * grp_per_batch - 12 + 2 * si, []
                    ).append((b, si * 1024, (si + 1) * 1024))

            mh16 = cpool.tile([C, M], F16)
            nc.sync.dma_start(out=mh16, in_=mh_in[:])
            m8 = cpool.tile([C, 4, M], F8)
            nc.sync.dma_start(out=m8, in_=m8_in[:])
            # batch 0: small leading slices so PE starts early
            for s0, s1 in ((0, 256), (256, 1024), (1024, 2048), (2048, 3072),
                           (3072, N)):
                load_slice(0, s0, s1)
            g8 = cpool.tile([TILE, KCH * 2 * C], F8)
            nc.sync.dma_start(out=g8, in_=g8_in[:])
            g8v = g8.rearrange("p (k h c) -> p k h c", k=KCH, h=2)
            idu = cpool.tile([TILE, TILE], F16)
            nc.sync.dma_start(out=idu, in_=idu_in[:])

            DR = mybir.MatmulPerfMode.DoubleRow

            def head(g):
                """Score matmuls for group g + per-tile negated max.
                Each tile gets its OWN 1-bank PSUM tile and reduce so the
                buffer-release semaphore fires per tile: head(g+2) tile j
                only waits on exp(g) tile j, keeping the PSUM-recycle cycle
                well under 2x the engine-busy period."""
                gtok0 = g * TILE * GRP
                b = gtok0 // N
                for lb, s0, s1 in load_plan.get(g, ()):
                    load_slice(lb, s0, s1)
                psts = []
                nbmxs = []
                for j in range(GRP):
                    tok0 = gtok0 + j * TILE
                    o = tok0 % LOAD
                    xht, x8t = xslice(b, o, TILE)
                    ps = ps_s.tile([TILE, M], F32, tag="pst")
                    nc.tensor.matmul(out=ps, lhsT=xht, rhs=mh16,
                                     start=True, stop=False)
                    nc.tensor.matmul(out=ps, lhsT=x8t[:, 0:2, :],
                                     rhs=m8[:, 0:2, :],
                                     start=False, stop=False, perf_mode=DR)
                    nc.tensor.matmul(out=ps, lhsT=x8t[:, 2:4, :],
                                     rhs=m8[:, 2:4, :],
                                     start=False, stop=True, perf_mode=DR)
                    nbmx = spool.tile([TILE, 1], F32, tag="nbmx")
                    nc.vector.reduce_max(
                        out=nbmx, in_=ps, axis=mybir.AxisListType.X,
                        negate=True,
                    )
                    psts.append(ps)
                    nbmxs.append(nbmx)
                return psts, nbmxs

            def exp_stage(g, psts, nbmxs):
                """Exp one-hot for group g (1 group late): both tiles write
                fp8 bytes into one shared fp16-typed pack tile (tile j owns
                byte plane j)."""
                ohp = ohpool.tile([TILE, M], F16)
                oh8 = ohp.bitcast(F8).rearrange("p (m two) -> p m two", two=2)
                for j in range(GRP):
                    nc.scalar.activation(
                        out=oh8[:, :, j], in_=psts[j], func=AF.Exp,
                        bias=nbmxs[j], scale=1.0,
                    )
                return ohp

            def tr_stage(g, ohp):
                """4 fp16 transposes move both byte planes at once (2 groups
                late) into a per-group 1-bank PSUM tile."""
                psT = ps_t.tile([TILE, KCH, TILE], F16, tag="psT")
                for k in range(KCH):
                    nc.tensor.transpose(
                        out=psT[:, k, :],
                        in_=ohp[:, k * TILE : (k + 1) * TILE],
                        identity=idu,
                    )
                return psT

            def copy_stage(psT, g):
                """Per-group copy (DVE 2x fp16 mode; every 8th on ACT to
                shave the DVE bottleneck) moves the transposed one-hots to
                SBUF, one iteration after the transposes. (GPSIMD cannot
                access PSUM, so DVE/ACT carry all copies.)"""
                ohT = ohtpool.tile([TILE, KCH, TILE], F16)
                if g % 4 == 0:
                    nc.scalar.activation(out=ohT, in_=psT, func=AF.Copy)
                else:
                    nc.vector.tensor_copy(ohT, psT)
                return ohT

            po_cur = [None]

            def gather_stage(g, ohT, half):
                """fp8 DoubleRow gather (3 groups late): per tile 4 DR
                matmuls (chunk-pair x hi/lo). Pairs of groups share one
                [C, 512] fp32 PSUM bank."""
                if g % 2 == 0:
                    po_tile = ps_o.tile([C, STORE], F32, tag="po")
                    po_cur[0] = po_tile
                po = po_cur[0]
                ohT8 = ohT.bitcast(F8).rearrange(
                    "p k (t two) -> p k t two", two=2
                )
                for j in range(GRP):
                    col0 = (g % 2) * GRP * TILE + j * TILE
                    n_mm = 0
                    for pr in range(KCH // 2):
                        for h in range(2):
                            nc.tensor.matmul(
                                out=po[:, col0 : col0 + TILE],
                                lhsT=g8v[:, 2 * pr : 2 * pr + 2, h, :],
                                rhs=ohT8[:, 2 * pr : 2 * pr + 2, :, j],
                                start=(n_mm == 0), stop=(n_mm == 2 * KCH - 1),
                                perf_mode=DR,
                            )
                            n_mm += 1
                return po if g % 2 == 1 else None

            def tail(g, po):
                """Batched out-copy on ACT (5 groups late): [C, 512] fp32
                PSUM -> fp16 SBUF, then one DMA per STORE tokens."""
                gtok0 = g * TILE * GRP
                ob = opool.tile([C, STORE], F16, tag="ob")
                nc.scalar.activation(out=ob, in_=po, func=AF.Copy)
                b, n0 = divmod(gtok0 + GRP * TILE - STORE, N)
                nc.sync.dma_start(out=out_d[b, :, n0 : n0 + STORE], in_=ob)

            # Software pipeline, one iteration per score group g:
            #   PE : tr(g-2) | gather(g-5) | head(g)     (ready work first;
            #        head's buf wait is the in-order SEQ block point)
            #   DVE: ohT copy (pair (g-4)//2) | reduce(g)
            #   ACT: exps(g-1) | tail copy(g-6)
            # The head->reduce->exp->head PSUM recycle is the critical cycle;
            # per-TILE subtile deps (exp tile j releases the score buf slice
            # for head(g+2) tile j) keep it under 2x the engine-busy period.
            p_head = None
            exps = {}
            psTs = {}
            ohts = {}
            pos = {}
            for g in range(n_groups + 8):
                # Per-iteration stages: head(g) | exp(g-1) | tr(g-2) |
                # copy(g-3) | gather(g-4) | tail(g-6, per pair). Every
                # cross-engine dependency is >=1 iteration old; the score-buf
                # recycle (head(g+2) tile j <- exp(g) tile j) is the
                # critical cycle, kept short by per-tile PSUM tiles.
                nxt_head = head(g) if g < n_groups else None
                g3 = g - 3
                if g3 >= 0 and g3 in psTs:
                    ohts[g3] = copy_stage(psTs.pop(g3), g3)
                if g - 2 >= 0 and (g - 2) in exps:
                    psTs[g - 2] = tr_stage(g - 2, exps.pop(g - 2))
                g4 = g - 4
                if g4 >= 0 and g4 in ohts:
                    po = gather_stage(g4, ohts.pop(g4), half=g4 % 2)
                    if po is not None:
                        pos[g4 // 2] = po
                if p_head is not None:
                    exps[g - 1] = exp_stage(g - 1, *p_head)
                g6 = g - 6
                if g6 >= 0 and g6 % 2 == 1 and (g6 // 2) in pos:
                    tail(g6, pos.pop(g6 // 2))
                p_head = nxt_head

    _legalize_waits(nc)
    nc.finalize()
    return nc


def _legalize_waits(nc):
    """This container's walrus accepts only ONE sync wait per engine
    instruction (setupSyncWait: 'Too many sync wait commands'). Tile emits
    multi-wait instructions (and an 11-wait tail drain). Split: keep one
    wait on the instruction, hoist the rest onto single-wait Drain ops
    inserted just before it on the same engine (engine order preserved =>
    semantics preserved). DMA copies are left alone (ring descriptors
    accept multiple waits)."""
    n_split = 0
    for f in nc.m.functions:
        for b in f.blocks:
            out = []
            for inst in b.instructions:
                si = inst.sync_info
                if si is not None and len(si.on_wait) > 1:
                    waits = list(si.on_wait)
                    for j, w in enumerate(waits[:-1]):
                        out.append(
                            mybir.InstDrain(
                                name=f"{inst.name}-w{j}",
                                engine=inst.engine,
                                ins=[],
                                outs=[],
                                sync_info=mybir.SyncInfo(
                                    on_wait=[w], on_update=[]
                                ),
                            )
                        )
                    inst.sync_info = mybir.SyncInfo(
                        on_wait=[waits[-1]], on_update=list(si.on_update)
                    )
                    n_split += 1
                out.append(inst)
            b.instructions = out
    return n_split


_NC = None


def _get_nc():
    global _NC
    if _NC is None:
        _NC = _build()
    return _NC


def _host_prep(x, memory):
    import ml_dtypes
    f8 = ml_dtypes.float8_e4m3

    memn = memory / np.maximum(
        np.sqrt((memory * memory).sum(axis=1, keepdims=True)), 1e-12
    )
    # Scale the normalized memory by BETA so PSUM holds beta*s directly
    # (exp sharpness K_eff = beta ~ 1e5). Cap keeps fp16 mh finite.
    beta = min(1e5, 55000.0 / max(float(np.abs(memn).max()), 1e-6))
    mnt = np.ascontiguousarray(memn.T).astype(np.float32) * beta   # [C, M]
    mh = mnt.astype(np.float16)
    ml = (mnt - mh.astype(np.float32)).astype(np.float32)

    # DR rhs slot tables (all exact power-of-2 shifts of fp8 encodings)
    mhf = mh.astype(np.float32)
    m8 = np.zeros((C, 4, M), dtype=f8)
    m8[:, 0, :] = (mhf / SIG).astype(f8)             # mhA
    m8[:, 1, :] = (mhf / (SIG * 16.0)).astype(f8)    # mhB
    m8[:, 2, :] = ml.astype(f8)                      # mlA
    m8[:, 3, :] = (ml / 16.0).astype(f8)             # mlB

    xh = x.astype(np.float16)
    xl = (x - xh.astype(np.float32)).astype(np.float32)
    xl1 = (xl * SIG).astype(f8)
    xl2 = ((xl * SIG - xl1.astype(np.float32)) * 16.0).astype(f8)
    xh8a = xh.astype(np.float32).astype(f8)
    xh8b = ((xh.astype(np.float32) - xh8a.astype(np.float32)) * 16.0).astype(f8)
    x8 = np.stack([xl1, xl2, xh8a, xh8b], axis=2)    # [B, C, 4, HW...]

    gh8 = memory.astype(f8)
    gl8 = (memory - gh8.astype(np.float32)).astype(f8)
    g8 = np.zeros((TILE, KCH * 2 * C), dtype=f8)
    for k in range(KCH):
        base = k * 2 * C
        g8[:, base : base + C] = gh8[k * TILE : (k + 1) * TILE, :]
        g8[:, base + C : base + 2 * C] = gl8[k * TILE : (k + 1) * TILE, :]

    return xh, x8, mh, m8, g8


def kernel(x, memory):
    x = np.asarray(x, dtype=np.float32)
    memory = np.asarray(memory, dtype=np.float32)
    nc = _get_nc()
    xf = x.reshape(B, C, N)
    xh, x8, mh16, m8, g8 = _host_prep(xf, memory)
    idu = np.eye(TILE, dtype=np.float16)

    in_maps = []
    for c in range(NCORES):
        in_maps.append({
            "xh": np.ascontiguousarray(xh[c * BPC : (c + 1) * BPC]),
            "x8": np.ascontiguousarray(x8[c * BPC : (c + 1) * BPC]),
            "mh16": mh16, "m8": m8, "g8": g8, "idu": idu,
        })

    res = run_bass_kernel_spmd(nc, in_maps, core_ids=list(range(NCORES)))
    outs = [
        r["out"].astype(np.float32).reshape(BPC, C, H, W) for r in res.results
    ]
    return np.concatenate(outs, axis=0)
